# revision 16
# baseline (speedup 1.0000x reference)
"""GCN (4-layer, PyG-default GCNConv) forward on 8 Trainium2 NeuronCores.

Strategy (node-parallel / graph-parallel):
  - Nodes are partitioned contiguously across the 8 cores (1250 rows each,
    padded to 1280 = 10 blocks of 128).
  - Per layer: each core computes its row-slice of G = H @ W as a tiled
    fp8-DoubleRow PE GEMM. G is quantized to fp8e4m3 and kept both in SBUF
    (per-parity resident tile; serves all LOCAL edges directly) and
    AllGathered in TWO pieces (producer blocks 0-3 -> "a", 4-9 -> "b"):
    AG-a fires after block 3's store, AG-b at layer end, so each collective
    overlaps the neighbouring layer's aggregation. (ncfw executes
    collectives serially at ~20-50us wall each, so fewer is better.)
  - Aggregation (symmetric-normalized adjacency incl. self-loops):
      * LOCAL edges (same-core source, incl. self-loops): dense per
        (src-block, dst-block) fp8 S_loc matrices [128, 128] folded into
        DoubleRow pairs against the SBUF-resident G tile -- no DMA at all.
        These matmuls fill the AllGather entry-latency window at each layer
        boundary.
      * REMOTE edges: fp8 source rows fetched from the gathered pieces with
        dma_gather (chunked calls, single-packet) and summed on the PE as
        OUT_block += S_chunk.T @ MSG_chunk, with S a host-built fp8
        [128e, 128d] weight matrix; consecutive chunks are paired into fp8
        DoubleRow matmuls. Rows are deduped per (source, dst-block).
  - Layer 4 output G4 = H4 @ W4 is aggregated at fp8 256-wide (2 classes
    padded; 256B gather rows) and log_softmax is fused on-chip.
"""

import sys

sys.path.insert(0, "/opt/trn_rl_repo")

import numpy as np
import ml_dtypes

BF16 = ml_dtypes.bfloat16
F8 = ml_dtypes.float8_e4m3

# Problem constants (nn_GCN_39195871543847)
N, E, F_IN, HID, C = 10000, 160000, 2208, 512, 2
W_CORES = 8
RPC = N // W_CORES  # 1250 nodes per core
MB = 10  # 128-row blocks per core
RPAD = MB * 128  # 1280
# Two AllGather pieces over the producer's padded rows (ncfw runs
# collectives serially at ~20-50us wall each regardless of payload, so
# fewer AGs wins; the "a" piece fires mid-layer, "b" at layer end).
NPIECE = 2
PIECE_ROWS = (512, 768)
PIECE_BASE = (0, 512)
KFC = (F_IN + 127) // 128  # 18 contraction chunks for layer 1
KFP = KFC * 128  # 2304
C_PAD = 256  # pad 2 output classes to 256 fp8 (256B gather rows)
CALL = 2  # 128-idx chunks per dma_gather call (384 idx = 25 descs/engine;
# small enough that 2 calls fit in a queue's descriptor ring, so the pool
# engine can run one call ahead per queue instead of stalling on reclaim)
N_QUEUES = 4  # SWDGE queues for gather descriptor generation

W1_SCALE = 32.0  # fp8 e4m3 min normal is 2^-6; glorot W1 needs upscaling
W_SCALE = 16.0   # same for W2/W3/W4


def _install_drain_patch():
    """This container's walrus accepts at most one sync-wait per instruction;
    TileContext's final drain gets one wait per live semaphore. Split the
    extra waits onto single-wait NOPs."""
    import bass_rust
    import concourse.tile as tile
    from concourse.vector_clock import ScopedClock

    if getattr(tile.TileContext, "_drain_patch_installed", False):
        return

    def _drain_and_barrier(self, tick_clock, wait_clock):
        drain_inst = self.nc.sync.drain()
        wait_clock.add_sem_waits(
            drain_inst.ins, ScopedClock({None: tick_clock.global_clock})
        )
        si = drain_inst.ins.sync_info
        waits = list(si.on_wait or []) if si is not None else []
        if len(waits) > 1:
            si.on_wait = waits[:1]
            for w in waits[1:]:
                nop = self.nc.sync.nop(nofuse=True)
                nop.ins.sync_info = bass_rust.SyncInfo(on_wait=[w], on_update=[])
        self.nc.all_engine_barrier()
        assert self.sems is not None
        popped = self.nc._tile_sem_poison_stack.pop()
        assert popped is self._sem_poison
        self.nc.clear_and_free_semaphores(list(self.sems.allocated().values()))
        self.nc.all_engine_barrier()

    tile.TileContext._drain_and_barrier = _drain_and_barrier
    tile.TileContext._drain_patch_installed = True


# ----------------------------------------------------------------------------
# Host-side graph preprocessing
# ----------------------------------------------------------------------------


def _preprocess(edge_index):
    """Per core: dense local S stack (per src-block x dst-block pair, incl.
    self-loops) + per (128-dst block, remote piece) deduped gather slots with
    their S stack and gather indices.

    Remote pieces: 0 = producer rows 0-511 ("a"), 1 = rows 512-1279
    ("b"). Chunk stream order (shared tb layout): [a b0..b9][b b0..b9]
    """
    src = edge_index[0].astype(np.int64)
    dst = edge_index[1].astype(np.int64)
    loop = np.arange(N, dtype=np.int64)
    s = np.concatenate([src, loop])
    d = np.concatenate([dst, loop])
    deg = np.bincount(d, minlength=N).astype(np.float32)
    dinv = np.where(deg > 0, 1.0 / np.sqrt(deg), 0.0).astype(np.float32)
    norm = (dinv[s] * dinv[d]).astype(np.float64)

    core = d // RPC
    slot_rows = {}
    edge_tuples = {}
    ka = np.zeros((MB, NPIECE), np.int64)
    sloc_np = []
    for c in range(W_CORES):
        m = core == c
        sc, dc, wc = s[m], d[m] - c * RPC, norm[m]
        s_core = sc // RPC
        s_loc = sc % RPC
        local = s_core == c

        # dense local S: [128(k=src pos), 100 (b*10+m), 128 (dst pos)]
        S_loc = np.zeros((128, MB * MB, 128), np.float32)
        lm = s_loc[local] // 128
        lk = s_loc[local] % 128
        lb = dc[local] // 128
        lj = dc[local] % 128
        np.add.at(S_loc, (lk, lb * MB + lm, lj), wc[local])
        sloc_np.append(S_loc.astype(F8))

        # remote classes
        rm = ~local
        scr, dcr, wcr = sc[rm], dc[rm], wc[rm]
        s_corer = scr // RPC
        s_locr = scr % RPC
        piece = np.where(s_locr < 512, 0, 1)
        g_row = np.zeros_like(s_locr)
        for h in range(NPIECE):
            mm = piece == h
            g_row[mm] = s_corer[mm] * PIECE_ROWS[h] + (s_locr[mm] - PIECE_BASE[h])
        blk = dcr // 128
        mloc = dcr % 128
        for b in range(MB):
            for h in range(NPIECE):
                mm = (blk == b) & (piece == h)
                rows = g_row[mm]
                ml = mloc[mm]
                ww = wcr[mm]
                uniq, inv = np.unique(rows, return_inverse=True)
                slot_rows[(c, b, h)] = uniq
                edge_tuples[(c, b, h)] = (inv, ml, ww)
                ka[b, h] = max(ka[b, h], 1, (len(uniq) + 127) // 128)

    seg_order = [(h, b) for h in range(NPIECE) for b in range(MB)]
    tb_off = {}
    t = 0
    for h, b in seg_order:
        tb_off[(b, h)] = t
        t += int(ka[b, h])
    T = t

    s_list, idx_list = [], []
    for c in range(W_CORES):
        S = np.zeros((T, 128, 128), np.float32)
        idx_flat = np.zeros(T * 128, np.int16)
        for h, b in seg_order:
            if (c, b, h) not in slot_rows:
                continue
            t0 = tb_off[(b, h)]
            uniq = slot_rows[(c, b, h)]
            inv, ml, ww = edge_tuples[(c, b, h)]
            k = np.arange(len(uniq))
            tt = t0 + k // 128
            kk = k % 128
            idx_flat[tt * 128 + kk] = uniq.astype(np.int16)
            np.add.at(S, (tt[inv], kk[inv], ml), ww)
        lay16 = idx_flat.reshape(T * 8, 16).T  # [16, T*8]
        idx_list.append(np.tile(lay16, (8, 1)).astype(np.int16))
        # SBUF-resident layout [128 partitions(k), T, 128(m)]
        s_list.append(
            np.ascontiguousarray(S.transpose(1, 0, 2)).astype(F8)
        )
    ka_t = tuple(
        (int(ka[b, 0]), int(ka[b, 1])) for b in range(MB)
    )
    return ka_t, s_list, idx_list, sloc_np


def _prep_inputs(x, edge_index, W1, b1, W2, b2, W3, b3, W4, b4):
    ka, s_list, idx_list, sloc_list = _preprocess(edge_index)

    # xT per core: [MB, 128(p), KFC, 128(j)]; xT[m,p,k,j] = x[c*RPC+m*128+j, k*128+p]
    xts = []
    for c in range(W_CORES):
        xp = np.zeros((RPAD, KFP), np.float32)
        xp[:RPC, :F_IN] = x[c * RPC : (c + 1) * RPC]
        xt = xp.reshape(MB, 128, KFC, 128).transpose(0, 3, 2, 1)
        xts.append(np.ascontiguousarray(xt).astype(F8))

    W1p = np.zeros((KFP, HID), np.float32)
    W1p[:F_IN] = W1 * W1_SCALE
    W1l = np.ascontiguousarray(
        W1p.reshape(KFC, 128, HID).transpose(1, 0, 2)
    ).astype(F8)
    W2l = np.ascontiguousarray(
        (W2 * W_SCALE).reshape(4, 128, HID).transpose(1, 0, 2)
    ).astype(F8)
    W3l = np.ascontiguousarray(
        (W3 * W_SCALE).reshape(4, 128, HID).transpose(1, 0, 2)
    ).astype(F8)
    W4p = np.zeros((HID, C_PAD), np.float32)
    W4p[:, :C] = W4 * W_SCALE
    W4l = np.ascontiguousarray(
        W4p.reshape(4, 128, C_PAD).transpose(1, 0, 2)
    ).astype(F8)

    b1r = np.broadcast_to(b1, (128, HID)).astype(np.float32).copy()
    b2r = np.broadcast_to(b2, (128, HID)).astype(np.float32).copy()
    b3r = np.broadcast_to(b3, (128, HID)).astype(np.float32).copy()
    b4r = np.zeros((128, C_PAD), np.float32)
    b4r[:, :C] = b4

    in_maps = []
    for c in range(W_CORES):
        in_maps.append(
            {
                "xT": xts[c],
                "W1l": W1l, "W2l": W2l, "W3l": W3l, "W4l": W4l,
                "b1r": b1r, "b2r": b2r, "b3r": b3r, "b4r": b4r,
                "S_in": s_list[c],
                "Sloc_in": sloc_list[c],
                "idx_in": idx_list[c],
            }
        )
    return ka, in_maps


# ----------------------------------------------------------------------------
# Bass kernel builder
# ----------------------------------------------------------------------------

_cache = {}


def _build(ka):
    import concourse.bass as bass
    import concourse.mybir as mybir
    from concourse.bacc import Bacc
    from concourse.tile import TileContext
    from concourse.masks import make_identity

    f32 = mybir.dt.float32
    bf16 = mybir.dt.bfloat16
    f8 = mybir.dt.float8e4
    i16 = mybir.dt.int16

    # chunk stream layout (must match _preprocess seg_order)
    seg_order = [(h, b) for h in range(NPIECE) for b in range(MB)]
    tb_off = {}
    t = 0
    for h, b in seg_order:
        tb_off[(b, h)] = t
        t += ka[b][h]
    T = t

    nc = Bacc(num_devices=W_CORES, num_swdge_queues=N_QUEUES)
    gq = [0]  # round-robin cursor over gather queues

    xT = nc.dram_tensor("xT", [MB, 128, KFC, 128], f8, kind="ExternalInput")
    W1l = nc.dram_tensor("W1l", [128, KFC, HID], f8, kind="ExternalInput")
    W2l = nc.dram_tensor("W2l", [128, 4, HID], f8, kind="ExternalInput")
    W3l = nc.dram_tensor("W3l", [128, 4, HID], f8, kind="ExternalInput")
    W4l = nc.dram_tensor("W4l", [128, 4, C_PAD], f8, kind="ExternalInput")
    b1r = nc.dram_tensor("b1r", [128, HID], f32, kind="ExternalInput")
    b2r = nc.dram_tensor("b2r", [128, HID], f32, kind="ExternalInput")
    b3r = nc.dram_tensor("b3r", [128, HID], f32, kind="ExternalInput")
    b4r = nc.dram_tensor("b4r", [128, C_PAD], f32, kind="ExternalInput")
    S_in = nc.dram_tensor("S_in", [128, T, 128], f8, kind="ExternalInput")
    Sloc_in = nc.dram_tensor(
        "Sloc_in", [128, MB * MB, 128], f8, kind="ExternalInput"
    )
    idx_in = nc.dram_tensor("idx_in", [128, T * 8], i16, kind="ExternalInput")
    out = nc.dram_tensor("out", [RPAD, C], f32, kind="ExternalOutput")

    # per-layer bounce pieces + gathered pieces (all fp8)
    l_wid = [HID, HID, HID, C_PAD]
    own_p = [[] for _ in range(NPIECE)]
    full_p = [[] for _ in range(NPIECE)]
    for l in range(4):
        for h in range(NPIECE):
            own_p[h].append(
                nc.dram_tensor(
                    f"own{'abc'[h]}{l}", [PIECE_ROWS[h], l_wid[l]], f8,
                    kind="Internal",
                )
            )
            full_p[h].append(
                nc.dram_tensor(
                    f"full{'abc'[h]}{l}",
                    [W_CORES * PIECE_ROWS[h], l_wid[l]], f8,
                    kind="Internal", addr_space="Shared",
                )
            )

    rg = [list(range(W_CORES))]

    # tiny dummy collective with no data deps: fires at t~0 and absorbs the
    # ncfw first-collective warmup (~60us) under the layer-1 GEMM
    warm_in = nc.dram_tensor("warm_in", [8, 64], f8, kind="Internal")
    warm_out = nc.dram_tensor(
        "warm_out", [64, 64], f8, kind="Internal", addr_space="Shared"
    )

    with TileContext(nc) as tc:
        with (
            tc.tile_pool(name="const", bufs=1) as cpool,
            tc.tile_pool(name="work", bufs=2) as wpool,
            tc.tile_pool(name="psum", bufs=2, space="PSUM") as ppool,
        ):
            relu = mybir.ActivationFunctionType.Relu
            copyf = mybir.ActivationFunctionType.Copy
            dbl = mybir.MatmulPerfMode.DoubleRow

            # index-count registers for the call windows (per block-class)
            sizes = {CALL * 128}
            for b in range(MB):
                for h in range(NPIECE):
                    if ka[b][h] % CALL:
                        sizes.add((ka[b][h] % CALL) * 128)
            r_cnt = {n: nc.gpsimd.to_reg(n) for n in sizes}

            # ---- layer-1 GEMM inputs first (critical path) -------------------
            W1_sb = []
            for k0 in range(0, KFC, 6):
                k1 = min(k0 + 6, KFC)
                wg = cpool.tile([128, k1 - k0, HID], f8, tag=f"w1g{k0}")
                nc.sync.dma_start(out=wg[:], in_=W1l[:, k0:k1, :])
                W1_sb.append(wg)

            # resident G tiles, one per parity: [128, MB, HID] fp8.
            # Layer l's GEMM writes gbt[l%2]; layer l's local aggregation
            # reads it. (Layer-3 G4 occupies [:, :, :C_PAD].)
            gbt = [
                cpool.tile(
                    [128, MB, HID], f8, tag=f"gbt{p}", name=f"gbt{p}"
                )
                for p in range(2)
            ]

            def allgather(own, full):
                nc.gpsimd.collective_compute(
                    "AllGather",
                    mybir.AluOpType.bypass,
                    ins=[own[:]],
                    outs=[full[:]],
                    replica_groups=rg,
                )

            def store_own(lslot, m, wid):
                """DMA gbt slice for block m to its own piece rows."""
                h = 0 if m < 4 else 1
                r0 = m * 128 - PIECE_BASE[h]
                nc.sync.dma_start(
                    out=own_p[h][lslot][r0 : r0 + 128, :],
                    in_=gbt[lslot % 2][:, m, :wid],
                )

            def gemm_l1():
                # fp8 DoubleRow: 9 k-pair matmuls per block.
                for m in range(MB):
                    xm = wpool.tile([128, KFC, 128], f8, tag="xm", bufs=3)
                    nc.sync.dma_start(out=xm[:], in_=xT[m])
                    ps = ppool.tile([128, HID], f32, tag="gps")
                    for k in range(0, KFC, 2):
                        nc.tensor.matmul(
                            ps[:],
                            lhsT=xm[:, k : k + 2, :],
                            rhs=W1_sb[k // 6][:, k % 6 : k % 6 + 2, :],
                            start=(k == 0),
                            stop=(k == KFC - 2),
                            perf_mode=dbl,
                        )
                    nc.scalar.activation(
                        gbt[0][:, m, :], ps[:], copyf, scale=1.0 / W1_SCALE
                    )
                    store_own(0, m, HID)
                    if m == 3:
                        allgather(own_p[0][0], full_p[0][0])
                    elif m == MB - 1:
                        allgather(own_p[1][0], full_p[1][0])

            nc.gpsimd.collective_compute(
                "AllGather",
                mybir.AluOpType.bypass,
                ins=[warm_in[:]],
                outs=[warm_out[:]],
                replica_groups=rg,
            )
            gemm_l1()

            # ---- remaining resident tensors (overlap the first collective) ---
            idx_sb = cpool.tile([128, T * 8], i16)
            nc.sync.dma_start(out=idx_sb[:], in_=idx_in[:])
            Sloc_sb = cpool.tile([128, MB * MB, 128], f8)
            nc.sync.dma_start(out=Sloc_sb[:], in_=Sloc_in[:])
            S_sb = cpool.tile([128, T, 128], f8)
            nc.sync.dma_start(out=S_sb[:], in_=S_in[:])
            W2_sb = cpool.tile([128, 4, HID], f8)
            nc.sync.dma_start(out=W2_sb[:], in_=W2l[:])
            W3_sb = cpool.tile([128, 4, HID], f8)
            nc.sync.dma_start(out=W3_sb[:], in_=W3l[:])
            W4_sb = cpool.tile([128, 4, C_PAD], f8)
            nc.sync.dma_start(out=W4_sb[:], in_=W4l[:])
            b_sb = []
            for nm, srcb in (("b1", b1r), ("b2", b2r), ("b3", b3r)):
                tle = cpool.tile([128, HID], f32, tag=f"bias_{nm}")
                nc.sync.dma_start(out=tle[:], in_=srcb[:])
                b_sb.append(tle)
            b4_sb = cpool.tile([128, C_PAD], f32)
            nc.sync.dma_start(out=b4_sb[:], in_=b4r[:])
            id_bf = cpool.tile([128, 128], bf16)
            make_identity(nc, id_bf[:])

            def issue_block_calls(b, h, fsrc, w, tiles):
                """Issue the gather calls for block b's piece-h chunk run;
                record tiles keyed by window-start chunk index."""
                t0 = tb_off[(b, h)]
                t1 = t0 + ka[b][h]
                for tw in range(t0, t1, CALL):
                    nk = min(CALL, t1 - tw)
                    msg = wpool.tile([128, CALL, w], f8, tag=f"msg{w}", bufs=24)
                    nc.gpsimd.dma_gather(
                        out_ap=msg[:, :nk, :],
                        in_ap=fsrc[:],
                        idxs_ap=idx_sb[:, tw * 8 : (tw + nk) * 8],
                        num_idxs=nk * 128,
                        num_idxs_reg=r_cnt[nk * 128],
                        elem_size=w,
                        queue_num=gq[0],
                    )
                    gq[0] = (gq[0] + 1) % N_QUEUES
                    tiles[tw] = msg

            def local_mms(ps, b, l, w, start):
                """Dense local aggregation for dst block b: 5 DoubleRow
                matmuls against the parity-resident G tile."""
                par = l % 2
                for m in range(0, MB, 2):
                    nc.tensor.matmul(
                        ps[:],
                        lhsT=Sloc_sb[:, b * MB + m : b * MB + m + 2, :],
                        rhs=gbt[par][:, m : m + 2, :w],
                        start=(start and m == 0),
                        stop=(m == MB - 2),
                        perf_mode=dbl,
                    )

            def half_mms(ps, b, hs, tiles, start, stop):
                """Accumulate block b's chunks for the pieces in hs into ps,
                pairing consecutive chunks within a call window (DoubleRow)."""
                runs = []  # (t, tw, pair)
                for h in hs:
                    t0 = tb_off[(b, h)]
                    t1 = t0 + ka[b][h]
                    for tw in range(t0, t1, CALL):
                        nk = min(CALL, t1 - tw)
                        i = 0
                        while i < nk:
                            if i + 1 < nk:
                                runs.append((tw + i, tw, True))
                                i += 2
                            else:
                                runs.append((tw + i, tw, False))
                                i += 1
                for i, (t, tw, pair) in enumerate(runs):
                    msg = tiles[tw]
                    if pair:
                        nc.tensor.matmul(
                            ps[:],
                            lhsT=S_sb[:, t : t + 2, :],
                            rhs=msg[:, t - tw : t - tw + 2, :],
                            start=(start and i == 0),
                            stop=(stop and i == len(runs) - 1),
                            perf_mode=dbl,
                        )
                    else:
                        nc.tensor.matmul(
                            ps[:],
                            lhsT=S_sb[:, t, :],
                            rhs=msg[:, t - tw, :],
                            start=(start and i == 0),
                            stop=(stop and i == len(runs) - 1),
                        )

            def logsoftmax_block(ps, hacc, m):
                lg0 = wpool.tile([128, C_PAD], f32, tag="lg0")
                nc.vector.tensor_add(out=lg0[:], in0=ps[:], in1=hacc[:])
                lg = wpool.tile([128, C_PAD], f32, tag="lg")
                nc.vector.tensor_add(out=lg[:], in0=lg0[:], in1=b4_sb[:])
                mx = wpool.tile([128, 1], f32, tag="mx")
                nc.vector.tensor_reduce(
                    out=mx[:], in_=lg[:, :C], axis=mybir.AxisListType.X,
                    op=mybir.AluOpType.max,
                )
                t2 = wpool.tile([128, C], f32, tag="t2")
                nc.vector.tensor_scalar(
                    out=t2[:], in0=lg[:, :C], scalar1=mx[:], scalar2=None,
                    op0=mybir.AluOpType.subtract,
                )
                e2 = wpool.tile([128, C], f32, tag="e2")
                nc.scalar.activation(e2[:], t2[:], mybir.ActivationFunctionType.Exp)
                sm = wpool.tile([128, 1], f32, tag="sm")
                nc.vector.tensor_reduce(
                    out=sm[:], in_=e2[:], axis=mybir.AxisListType.X,
                    op=mybir.AluOpType.add,
                )
                ls = wpool.tile([128, 1], f32, tag="ls")
                nc.scalar.activation(ls[:], sm[:], mybir.ActivationFunctionType.Ln)
                o2 = wpool.tile([128, C], f32, tag="o2")
                nc.vector.tensor_scalar(
                    out=o2[:], in0=t2[:], scalar1=ls[:], scalar2=None,
                    op0=mybir.AluOpType.subtract,
                )
                nc.sync.dma_start(out=out[m * 128 : (m + 1) * 128, :], in_=o2[:])

            def block_tail(l, b, ps, hacc, bias_t, mode):
                """Bias + phase-A partial + relu + transpose + next-layer GEMM
                + store for one finished block (or the final classifier)."""
                if mode[0] == "final":
                    logsoftmax_block(ps, hacc, b)
                    return
                h0 = wpool.tile([128, HID], f32, tag="h0", bufs=3)
                nc.vector.tensor_add(out=h0[:], in0=ps[:], in1=hacc[:])
                hf = wpool.tile([128, HID], f32, tag="hf", bufs=3)
                nc.vector.tensor_add(out=hf[:], in0=h0[:], in1=bias_t[:])
                hb = wpool.tile([128, HID], bf16, tag="hb", bufs=3)
                nc.scalar.activation(hb[:], hf[:], relu)
                ht = wpool.tile([128, 4, 128], f8, tag="ht", bufs=4)
                for gg in range(4):
                    tp = ppool.tile([128, 128], bf16, tag="tps", bufs=1)
                    nc.tensor.transpose(
                        tp[:], hb[:, gg * 128 : (gg + 1) * 128], id_bf[:]
                    )
                    nc.vector.tensor_copy(out=ht[:, gg, :], in_=tp[:])
                _, w_sb, lnext = mode
                wid = l_wid[lnext]
                gp = ppool.tile([128, wid], f32, tag="gps")
                for k in range(0, 4, 2):
                    nc.tensor.matmul(
                        gp[:],
                        lhsT=ht[:, k : k + 2, :],
                        rhs=w_sb[:, k : k + 2, :],
                        start=(k == 0),
                        stop=(k == 2),
                        perf_mode=dbl,
                    )
                nc.scalar.activation(
                    gbt[lnext % 2][:, b, :wid], gp[:], copyf,
                    scale=1.0 / W_SCALE,
                )
                store_own(lnext, b, wid)

            def layer(l, bias_t, mode):
                """One fused layer: phase A accumulates local (SBUF-direct)
                + piece-a chunks into PSUM and spills; phase B adds piece-b
                and runs the per-block tail. The next layer's piece-a AG
                fires from tail 3, piece-b at layer end."""
                w = l_wid[l]
                pw = HID if mode[0] != "final" else C_PAD
                # local pass first: pure-PE work with no DMA deps beyond the
                # previous layer's tails -- fills the AG/boundary window.
                # (Reuses the apsB PSUM buffers, which are idle here.)
                loc_acc = {}
                for b in range(MB):
                    psL = ppool.tile([128, pw], f32, tag="apsB", bufs=2)
                    local_mms(psL, b, l, w, True)
                    hl = wpool.tile([128, pw], bf16, tag="haccL", bufs=10)
                    nc.vector.tensor_copy(out=hl[:], in_=psL[:])
                    loc_acc[b] = hl
                tiles = {}
                for b in range(MB):
                    issue_block_calls(b, 0, full_p[0][l], w, tiles)
                haccs = {}
                for b in range(MB):
                    psA = ppool.tile([128, pw], f32, tag="apsA", bufs=2)
                    half_mms(psA, b, (0,), tiles, True, True)
                    # fold the local partial into the bf16 spill (same DVE op)
                    hc = wpool.tile([128, pw], bf16, tag="hacc", bufs=10)
                    nc.vector.tensor_add(out=hc[:], in0=psA[:], in1=loc_acc[b][:])
                    haccs[b] = hc
                for b in range(MB):
                    issue_block_calls(b, 1, full_p[1][l], w, tiles)
                for b in range(MB):
                    psB = ppool.tile([128, pw], f32, tag="apsB", bufs=2)
                    half_mms(psB, b, (1,), tiles, True, True)
                    block_tail(l, b, psB, haccs[b], bias_t, mode)
                    if mode[0] != "final" and b == 3:
                        allgather(own_p[0][mode[2]], full_p[0][mode[2]])
                if mode[0] != "final":
                    allgather(own_p[1][mode[2]], full_p[1][mode[2]])

            # ---- layers ----------------------------------------------------
            layer(0, b_sb[0], ("gemm", W2_sb, 1))
            layer(1, b_sb[1], ("gemm", W3_sb, 2))
            layer(2, b_sb[2], ("gemm", W4_sb, 3))
            layer(3, None, ("final",))

    nc.compile()
    return nc


# ----------------------------------------------------------------------------
# Entry point
# ----------------------------------------------------------------------------


def kernel(x, edge_index, batch, W1, b1, W2, b2, W3, b3, W4, b4, _trace=False):
    _install_drain_patch()
    from concourse.bass_utils import run_bass_kernel_spmd

    ka, in_maps = _prep_inputs(
        np.asarray(x, np.float32),
        np.asarray(edge_index),
        np.asarray(W1, np.float32), np.asarray(b1, np.float32),
        np.asarray(W2, np.float32), np.asarray(b2, np.float32),
        np.asarray(W3, np.float32), np.asarray(b3, np.float32),
        np.asarray(W4, np.float32), np.asarray(b4, np.float32),
    )
    key = tuple(ka)
    if key not in _cache:
        _cache[key] = _build(ka)
    nc = _cache[key]
    res = run_bass_kernel_spmd(
        nc, in_maps, core_ids=list(range(W_CORES)), trace=_trace
    )
    outp = np.concatenate(
        [res.results[c]["out"][:RPC] for c in range(W_CORES)], axis=0
    ).astype(np.float32)
    if _trace:
        return outp, res
    return outp


# revision 17
# speedup vs baseline: 1.0235x; 1.0235x over previous
"""GCN (4-layer, PyG-default GCNConv) forward on 8 Trainium2 NeuronCores.

Strategy (node-parallel / graph-parallel):
  - Nodes are partitioned contiguously across the 8 cores (1250 rows each,
    padded to 1280 = 10 blocks of 128).
  - Per layer: each core computes its row-slice of G = H @ W as a tiled
    fp8-DoubleRow PE GEMM. G is quantized to fp8e4m3 and kept both in SBUF
    (per-parity resident tile; serves all LOCAL edges directly) and
    AllGathered in TWO pieces (producer blocks 0-3 -> "a", 4-9 -> "b"):
    AG-a fires after block 3's store, AG-b at layer end, so each collective
    overlaps the neighbouring layer's aggregation. (ncfw executes
    collectives serially at ~20-50us wall each, so fewer is better.)
  - Aggregation (symmetric-normalized adjacency incl. self-loops):
      * LOCAL edges (same-core source, incl. self-loops): dense per
        (src-block, dst-block) fp8 S_loc matrices [128, 128] folded into
        DoubleRow pairs against the SBUF-resident G tile -- no DMA at all.
        These matmuls fill the AllGather entry-latency window at each layer
        boundary.
      * REMOTE edges: fp8 source rows fetched from the gathered pieces with
        dma_gather (chunked calls, single-packet) and summed on the PE as
        OUT_block += S_chunk.T @ MSG_chunk, with S a host-built fp8
        [128e, 128d] weight matrix; consecutive chunks are paired into fp8
        DoubleRow matmuls. Rows are deduped per (source, dst-block).
  - Layer 4 output G4 = H4 @ W4 is aggregated at fp8 256-wide (2 classes
    padded; 256B gather rows) and log_softmax is fused on-chip.
"""

import sys

sys.path.insert(0, "/opt/trn_rl_repo")

import numpy as np
import ml_dtypes

BF16 = ml_dtypes.bfloat16
F8 = ml_dtypes.float8_e4m3

# Problem constants (nn_GCN_39195871543847)
N, E, F_IN, HID, C = 10000, 160000, 2208, 512, 2
W_CORES = 8
RPC = N // W_CORES  # 1250 nodes per core
MB = 10  # 128-row blocks per core
RPAD = MB * 128  # 1280
# Two AllGather pieces over the producer's padded rows (ncfw runs
# collectives serially at ~20-50us wall each regardless of payload, so
# fewer AGs wins; the "a" piece fires mid-layer, "b" at layer end).
NPIECE = 2
PIECE_ROWS = (512, 768)
PIECE_BASE = (0, 512)
KFC = (F_IN + 127) // 128  # 18 contraction chunks for layer 1
KFP = KFC * 128  # 2304
C_PAD = 256  # pad 2 output classes to 256 fp8 (256B gather rows)
CALL = 2  # 128-idx chunks per dma_gather call (384 idx = 25 descs/engine;
# small enough that 2 calls fit in a queue's descriptor ring, so the pool
# engine can run one call ahead per queue instead of stalling on reclaim)
N_QUEUES = 4  # SWDGE queues for gather descriptor generation

W1_SCALE = 32.0  # fp8 e4m3 min normal is 2^-6; glorot W1 needs upscaling
W_SCALE = 16.0   # same for W2/W3/W4


def _install_drain_patch():
    """This container's walrus accepts at most one sync-wait per instruction;
    TileContext's final drain gets one wait per live semaphore. Split the
    extra waits onto single-wait NOPs."""
    import bass_rust
    import concourse.tile as tile
    from concourse.vector_clock import ScopedClock

    if getattr(tile.TileContext, "_drain_patch_installed", False):
        return

    def _drain_and_barrier(self, tick_clock, wait_clock):
        drain_inst = self.nc.sync.drain()
        wait_clock.add_sem_waits(
            drain_inst.ins, ScopedClock({None: tick_clock.global_clock})
        )
        si = drain_inst.ins.sync_info
        waits = list(si.on_wait or []) if si is not None else []
        if len(waits) > 1:
            si.on_wait = waits[:1]
            for w in waits[1:]:
                nop = self.nc.sync.nop(nofuse=True)
                nop.ins.sync_info = bass_rust.SyncInfo(on_wait=[w], on_update=[])
        self.nc.all_engine_barrier()
        assert self.sems is not None
        popped = self.nc._tile_sem_poison_stack.pop()
        assert popped is self._sem_poison
        self.nc.clear_and_free_semaphores(list(self.sems.allocated().values()))
        self.nc.all_engine_barrier()

    tile.TileContext._drain_and_barrier = _drain_and_barrier
    tile.TileContext._drain_patch_installed = True


# ----------------------------------------------------------------------------
# Host-side graph preprocessing
# ----------------------------------------------------------------------------


def _preprocess(edge_index):
    """Per core: dense local S stack (per src-block x dst-block pair, incl.
    self-loops) + per (128-dst block, remote piece) deduped gather slots with
    their S stack and gather indices.

    Remote pieces: 0 = producer rows 0-511 ("a"), 1 = rows 512-1279
    ("b"). Chunk stream order (shared tb layout): [a b0..b9][b b0..b9]
    """
    src = edge_index[0].astype(np.int64)
    dst = edge_index[1].astype(np.int64)
    loop = np.arange(N, dtype=np.int64)
    s = np.concatenate([src, loop])
    d = np.concatenate([dst, loop])
    deg = np.bincount(d, minlength=N).astype(np.float32)
    dinv = np.where(deg > 0, 1.0 / np.sqrt(deg), 0.0).astype(np.float32)
    norm = (dinv[s] * dinv[d]).astype(np.float64)

    core = d // RPC
    slot_rows = {}
    edge_tuples = {}
    ka = np.zeros((MB, NPIECE), np.int64)
    sloc_np = []
    for c in range(W_CORES):
        m = core == c
        sc, dc, wc = s[m], d[m] - c * RPC, norm[m]
        s_core = sc // RPC
        s_loc = sc % RPC
        local = s_core == c

        # dense local S: [128(k=src pos), 100 (b*10+m), 128 (dst pos)]
        S_loc = np.zeros((128, MB * MB, 128), np.float32)
        lm = s_loc[local] // 128
        lk = s_loc[local] % 128
        lb = dc[local] // 128
        lj = dc[local] % 128
        np.add.at(S_loc, (lk, lb * MB + lm, lj), wc[local])
        sloc_np.append(S_loc.astype(F8))

        # remote classes
        rm = ~local
        scr, dcr, wcr = sc[rm], dc[rm], wc[rm]
        s_corer = scr // RPC
        s_locr = scr % RPC
        piece = np.where(s_locr < 512, 0, 1)
        g_row = np.zeros_like(s_locr)
        for h in range(NPIECE):
            mm = piece == h
            g_row[mm] = s_corer[mm] * PIECE_ROWS[h] + (s_locr[mm] - PIECE_BASE[h])
        blk = dcr // 128
        mloc = dcr % 128
        for b in range(MB):
            for h in range(NPIECE):
                mm = (blk == b) & (piece == h)
                rows = g_row[mm]
                ml = mloc[mm]
                ww = wcr[mm]
                uniq, inv = np.unique(rows, return_inverse=True)
                slot_rows[(c, b, h)] = uniq
                edge_tuples[(c, b, h)] = (inv, ml, ww)
                ka[b, h] = max(ka[b, h], 1, (len(uniq) + 127) // 128)

    seg_order = [(h, b) for h in range(NPIECE) for b in range(MB)]
    tb_off = {}
    t = 0
    for h, b in seg_order:
        tb_off[(b, h)] = t
        t += int(ka[b, h])
    T = t

    s_list, idx_list = [], []
    for c in range(W_CORES):
        S = np.zeros((T, 128, 128), np.float32)
        idx_flat = np.zeros(T * 128, np.int16)
        for h, b in seg_order:
            if (c, b, h) not in slot_rows:
                continue
            t0 = tb_off[(b, h)]
            uniq = slot_rows[(c, b, h)]
            inv, ml, ww = edge_tuples[(c, b, h)]
            k = np.arange(len(uniq))
            tt = t0 + k // 128
            kk = k % 128
            idx_flat[tt * 128 + kk] = uniq.astype(np.int16)
            np.add.at(S, (tt[inv], kk[inv], ml), ww)
        lay16 = idx_flat.reshape(T * 8, 16).T  # [16, T*8]
        idx_list.append(np.tile(lay16, (8, 1)).astype(np.int16))
        # SBUF-resident layout [128 partitions(k), T, 128(m)]
        s_list.append(
            np.ascontiguousarray(S.transpose(1, 0, 2)).astype(F8)
        )
    ka_t = tuple(
        (int(ka[b, 0]), int(ka[b, 1])) for b in range(MB)
    )
    return ka_t, s_list, idx_list, sloc_np


def _prep_inputs(x, edge_index, W1, b1, W2, b2, W3, b3, W4, b4):
    ka, s_list, idx_list, sloc_list = _preprocess(edge_index)

    # xT per core: [MB, 128(p), KFC, 128(j)]; xT[m,p,k,j] = x[c*RPC+m*128+j, k*128+p]
    xts = []
    for c in range(W_CORES):
        xp = np.zeros((RPAD, KFP), np.float32)
        xp[:RPC, :F_IN] = x[c * RPC : (c + 1) * RPC]
        xt = xp.reshape(MB, 128, KFC, 128).transpose(0, 3, 2, 1)
        xts.append(np.ascontiguousarray(xt).astype(F8))

    W1p = np.zeros((KFP, HID), np.float32)
    W1p[:F_IN] = W1 * W1_SCALE
    W1l = np.ascontiguousarray(
        W1p.reshape(KFC, 128, HID).transpose(1, 0, 2)
    ).astype(F8)
    W2l = np.ascontiguousarray(
        (W2 * W_SCALE).reshape(4, 128, HID).transpose(1, 0, 2)
    ).astype(F8)
    W3l = np.ascontiguousarray(
        (W3 * W_SCALE).reshape(4, 128, HID).transpose(1, 0, 2)
    ).astype(F8)
    W4p = np.zeros((HID, C_PAD), np.float32)
    W4p[:, :C] = W4 * W_SCALE
    W4l = np.ascontiguousarray(
        W4p.reshape(4, 128, C_PAD).transpose(1, 0, 2)
    ).astype(F8)

    b1r = np.broadcast_to(b1, (128, HID)).astype(np.float32).copy()
    b2r = np.broadcast_to(b2, (128, HID)).astype(np.float32).copy()
    b3r = np.broadcast_to(b3, (128, HID)).astype(np.float32).copy()
    b4r = np.zeros((128, C_PAD), np.float32)
    b4r[:, :C] = b4

    in_maps = []
    for c in range(W_CORES):
        in_maps.append(
            {
                "xT": xts[c],
                "W1l": W1l, "W2l": W2l, "W3l": W3l, "W4l": W4l,
                "b1r": b1r, "b2r": b2r, "b3r": b3r, "b4r": b4r,
                "S_in": s_list[c],
                "Sloc_in": sloc_list[c],
                "idx_in": idx_list[c],
            }
        )
    return ka, in_maps


# ----------------------------------------------------------------------------
# Bass kernel builder
# ----------------------------------------------------------------------------

_cache = {}


def _build(ka):
    import concourse.bass as bass
    import concourse.mybir as mybir
    from concourse.bacc import Bacc
    from concourse.tile import TileContext
    from concourse.masks import make_identity

    f32 = mybir.dt.float32
    bf16 = mybir.dt.bfloat16
    f8 = mybir.dt.float8e4
    i16 = mybir.dt.int16

    # chunk stream layout (must match _preprocess seg_order)
    seg_order = [(h, b) for h in range(NPIECE) for b in range(MB)]
    tb_off = {}
    t = 0
    for h, b in seg_order:
        tb_off[(b, h)] = t
        t += ka[b][h]
    T = t

    nc = Bacc(num_devices=W_CORES, num_swdge_queues=N_QUEUES)
    gq = [0]  # round-robin cursor over gather queues

    xT = nc.dram_tensor("xT", [MB, 128, KFC, 128], f8, kind="ExternalInput")
    W1l = nc.dram_tensor("W1l", [128, KFC, HID], f8, kind="ExternalInput")
    W2l = nc.dram_tensor("W2l", [128, 4, HID], f8, kind="ExternalInput")
    W3l = nc.dram_tensor("W3l", [128, 4, HID], f8, kind="ExternalInput")
    W4l = nc.dram_tensor("W4l", [128, 4, C_PAD], f8, kind="ExternalInput")
    b1r = nc.dram_tensor("b1r", [128, HID], f32, kind="ExternalInput")
    b2r = nc.dram_tensor("b2r", [128, HID], f32, kind="ExternalInput")
    b3r = nc.dram_tensor("b3r", [128, HID], f32, kind="ExternalInput")
    b4r = nc.dram_tensor("b4r", [128, C_PAD], f32, kind="ExternalInput")
    S_in = nc.dram_tensor("S_in", [128, T, 128], f8, kind="ExternalInput")
    Sloc_in = nc.dram_tensor(
        "Sloc_in", [128, MB * MB, 128], f8, kind="ExternalInput"
    )
    idx_in = nc.dram_tensor("idx_in", [128, T * 8], i16, kind="ExternalInput")
    out = nc.dram_tensor("out", [RPAD, C], f32, kind="ExternalOutput")

    # per-layer bounce pieces + gathered pieces (all fp8)
    l_wid = [HID, HID, HID, C_PAD]
    own_p = [[] for _ in range(NPIECE)]
    full_p = [[] for _ in range(NPIECE)]
    for l in range(4):
        for h in range(NPIECE):
            own_p[h].append(
                nc.dram_tensor(
                    f"own{'abc'[h]}{l}", [PIECE_ROWS[h], l_wid[l]], f8,
                    kind="Internal",
                )
            )
            full_p[h].append(
                nc.dram_tensor(
                    f"full{'abc'[h]}{l}",
                    [W_CORES * PIECE_ROWS[h], l_wid[l]], f8,
                    kind="Internal", addr_space="Shared",
                )
            )

    rg = [list(range(W_CORES))]

    # tiny dummy collective with no data deps: fires at t~0 and absorbs the
    # ncfw first-collective warmup (~60us) under the layer-1 GEMM
    warm_in = nc.dram_tensor("warm_in", [8, 64], f8, kind="Internal")
    warm_out = nc.dram_tensor(
        "warm_out", [64, 64], f8, kind="Internal", addr_space="Shared"
    )

    with TileContext(nc) as tc:
        with (
            tc.tile_pool(name="const", bufs=1) as cpool,
            tc.tile_pool(name="work", bufs=2) as wpool,
            tc.tile_pool(name="psum", bufs=2, space="PSUM") as ppool,
        ):
            relu = mybir.ActivationFunctionType.Relu
            copyf = mybir.ActivationFunctionType.Copy
            dbl = mybir.MatmulPerfMode.DoubleRow

            # index-count registers for the call windows (per block-class)
            sizes = {CALL * 128}
            for b in range(MB):
                for h in range(NPIECE):
                    if ka[b][h] % CALL:
                        sizes.add((ka[b][h] % CALL) * 128)
            r_cnt = {n: nc.gpsimd.to_reg(n) for n in sizes}

            # ---- layer-1 GEMM inputs first (critical path) -------------------
            W1_sb = []
            for k0 in range(0, KFC, 6):
                k1 = min(k0 + 6, KFC)
                wg = cpool.tile([128, k1 - k0, HID], f8, tag=f"w1g{k0}")
                nc.sync.dma_start(out=wg[:], in_=W1l[:, k0:k1, :])
                W1_sb.append(wg)

            # resident G tiles, one per parity: [128, MB, HID] fp8.
            # Layer l's GEMM writes gbt[l%2]; layer l's local aggregation
            # reads it. (Layer-3 G4 occupies [:, :, :C_PAD].)
            gbt = [
                cpool.tile(
                    [128, MB, HID], f8, tag=f"gbt{p}", name=f"gbt{p}"
                )
                for p in range(2)
            ]

            def allgather(own, full):
                nc.gpsimd.collective_compute(
                    "AllGather",
                    mybir.AluOpType.bypass,
                    ins=[own[:]],
                    outs=[full[:]],
                    replica_groups=rg,
                )

            def store_own(lslot, m, wid):
                """DMA gbt slice for block m to its own piece rows."""
                h = 0 if m < 4 else 1
                r0 = m * 128 - PIECE_BASE[h]
                nc.sync.dma_start(
                    out=own_p[h][lslot][r0 : r0 + 128, :],
                    in_=gbt[lslot % 2][:, m, :wid],
                )

            def gemm_l1():
                # fp8 DoubleRow: 9 k-pair matmuls per block.
                for m in range(MB):
                    xm = wpool.tile([128, KFC, 128], f8, tag="xm", bufs=3)
                    nc.sync.dma_start(out=xm[:], in_=xT[m])
                    ps = ppool.tile([128, HID], f32, tag="gps")
                    for k in range(0, KFC, 2):
                        nc.tensor.matmul(
                            ps[:],
                            lhsT=xm[:, k : k + 2, :],
                            rhs=W1_sb[k // 6][:, k % 6 : k % 6 + 2, :],
                            start=(k == 0),
                            stop=(k == KFC - 2),
                            perf_mode=dbl,
                        )
                    nc.scalar.activation(
                        gbt[0][:, m, :], ps[:], copyf, scale=1.0 / W1_SCALE
                    )
                    store_own(0, m, HID)
                    if m == 3:
                        allgather(own_p[0][0], full_p[0][0])
                    elif m == MB - 1:
                        allgather(own_p[1][0], full_p[1][0])

            gemm_l1()

            # ---- remaining resident tensors (overlap the first collective) ---
            idx_sb = cpool.tile([128, T * 8], i16)
            nc.sync.dma_start(out=idx_sb[:], in_=idx_in[:])
            Sloc_sb = cpool.tile([128, MB * MB, 128], f8)
            nc.sync.dma_start(out=Sloc_sb[:], in_=Sloc_in[:])
            S_sb = cpool.tile([128, T, 128], f8)
            nc.sync.dma_start(out=S_sb[:], in_=S_in[:])
            W2_sb = cpool.tile([128, 4, HID], f8)
            nc.sync.dma_start(out=W2_sb[:], in_=W2l[:])
            W3_sb = cpool.tile([128, 4, HID], f8)
            nc.sync.dma_start(out=W3_sb[:], in_=W3l[:])
            W4_sb = cpool.tile([128, 4, C_PAD], f8)
            nc.sync.dma_start(out=W4_sb[:], in_=W4l[:])
            b_sb = []
            for nm, srcb in (("b1", b1r), ("b2", b2r), ("b3", b3r)):
                tle = cpool.tile([128, HID], f32, tag=f"bias_{nm}")
                nc.sync.dma_start(out=tle[:], in_=srcb[:])
                b_sb.append(tle)
            b4_sb = cpool.tile([128, C_PAD], f32)
            nc.sync.dma_start(out=b4_sb[:], in_=b4r[:])
            id_bf = cpool.tile([128, 128], bf16)
            make_identity(nc, id_bf[:])

            def issue_block_calls(b, h, fsrc, w, tiles):
                """Issue the gather calls for block b's piece-h chunk run;
                record tiles keyed by window-start chunk index."""
                t0 = tb_off[(b, h)]
                t1 = t0 + ka[b][h]
                for tw in range(t0, t1, CALL):
                    nk = min(CALL, t1 - tw)
                    msg = wpool.tile([128, CALL, w], f8, tag=f"msg{w}", bufs=24)
                    nc.gpsimd.dma_gather(
                        out_ap=msg[:, :nk, :],
                        in_ap=fsrc[:],
                        idxs_ap=idx_sb[:, tw * 8 : (tw + nk) * 8],
                        num_idxs=nk * 128,
                        num_idxs_reg=r_cnt[nk * 128],
                        elem_size=w,
                        queue_num=gq[0],
                    )
                    gq[0] = (gq[0] + 1) % N_QUEUES
                    tiles[tw] = msg

            def local_mms(ps, b, l, w, start):
                """Dense local aggregation for dst block b: 5 DoubleRow
                matmuls against the parity-resident G tile."""
                par = l % 2
                for m in range(0, MB, 2):
                    nc.tensor.matmul(
                        ps[:],
                        lhsT=Sloc_sb[:, b * MB + m : b * MB + m + 2, :],
                        rhs=gbt[par][:, m : m + 2, :w],
                        start=(start and m == 0),
                        stop=(m == MB - 2),
                        perf_mode=dbl,
                    )

            def half_mms(ps, b, hs, tiles, start, stop):
                """Accumulate block b's chunks for the pieces in hs into ps,
                pairing consecutive chunks within a call window (DoubleRow)."""
                runs = []  # (t, tw, pair)
                for h in hs:
                    t0 = tb_off[(b, h)]
                    t1 = t0 + ka[b][h]
                    for tw in range(t0, t1, CALL):
                        nk = min(CALL, t1 - tw)
                        i = 0
                        while i < nk:
                            if i + 1 < nk:
                                runs.append((tw + i, tw, True))
                                i += 2
                            else:
                                runs.append((tw + i, tw, False))
                                i += 1
                for i, (t, tw, pair) in enumerate(runs):
                    msg = tiles[tw]
                    if pair:
                        nc.tensor.matmul(
                            ps[:],
                            lhsT=S_sb[:, t : t + 2, :],
                            rhs=msg[:, t - tw : t - tw + 2, :],
                            start=(start and i == 0),
                            stop=(stop and i == len(runs) - 1),
                            perf_mode=dbl,
                        )
                    else:
                        nc.tensor.matmul(
                            ps[:],
                            lhsT=S_sb[:, t, :],
                            rhs=msg[:, t - tw, :],
                            start=(start and i == 0),
                            stop=(stop and i == len(runs) - 1),
                        )

            def logsoftmax_block(ps, hacc, m):
                lg0 = wpool.tile([128, C_PAD], f32, tag="lg0")
                nc.vector.tensor_add(out=lg0[:], in0=ps[:], in1=hacc[:])
                lg = wpool.tile([128, C_PAD], f32, tag="lg")
                nc.vector.tensor_add(out=lg[:], in0=lg0[:], in1=b4_sb[:])
                mx = wpool.tile([128, 1], f32, tag="mx")
                nc.vector.tensor_reduce(
                    out=mx[:], in_=lg[:, :C], axis=mybir.AxisListType.X,
                    op=mybir.AluOpType.max,
                )
                t2 = wpool.tile([128, C], f32, tag="t2")
                nc.vector.tensor_scalar(
                    out=t2[:], in0=lg[:, :C], scalar1=mx[:], scalar2=None,
                    op0=mybir.AluOpType.subtract,
                )
                e2 = wpool.tile([128, C], f32, tag="e2")
                nc.scalar.activation(e2[:], t2[:], mybir.ActivationFunctionType.Exp)
                sm = wpool.tile([128, 1], f32, tag="sm")
                nc.vector.tensor_reduce(
                    out=sm[:], in_=e2[:], axis=mybir.AxisListType.X,
                    op=mybir.AluOpType.add,
                )
                ls = wpool.tile([128, 1], f32, tag="ls")
                nc.scalar.activation(ls[:], sm[:], mybir.ActivationFunctionType.Ln)
                o2 = wpool.tile([128, C], f32, tag="o2")
                nc.vector.tensor_scalar(
                    out=o2[:], in0=t2[:], scalar1=ls[:], scalar2=None,
                    op0=mybir.AluOpType.subtract,
                )
                nc.sync.dma_start(out=out[m * 128 : (m + 1) * 128, :], in_=o2[:])

            def block_tail(l, b, ps, hacc, bias_t, mode):
                """Bias + phase-A partial + relu + transpose + next-layer GEMM
                + store for one finished block (or the final classifier)."""
                if mode[0] == "final":
                    logsoftmax_block(ps, hacc, b)
                    return
                h0 = wpool.tile([128, HID], f32, tag="h0", bufs=3)
                nc.vector.tensor_add(out=h0[:], in0=ps[:], in1=hacc[:])
                hf = wpool.tile([128, HID], f32, tag="hf", bufs=3)
                nc.vector.tensor_add(out=hf[:], in0=h0[:], in1=bias_t[:])
                hb = wpool.tile([128, HID], bf16, tag="hb", bufs=3)
                nc.scalar.activation(hb[:], hf[:], relu)
                ht = wpool.tile([128, 4, 128], f8, tag="ht", bufs=4)
                for gg in range(4):
                    tp = ppool.tile([128, 128], bf16, tag="tps", bufs=1)
                    nc.tensor.transpose(
                        tp[:], hb[:, gg * 128 : (gg + 1) * 128], id_bf[:]
                    )
                    nc.vector.tensor_copy(out=ht[:, gg, :], in_=tp[:])
                _, w_sb, lnext = mode
                wid = l_wid[lnext]
                gp = ppool.tile([128, wid], f32, tag="gps")
                for k in range(0, 4, 2):
                    nc.tensor.matmul(
                        gp[:],
                        lhsT=ht[:, k : k + 2, :],
                        rhs=w_sb[:, k : k + 2, :],
                        start=(k == 0),
                        stop=(k == 2),
                        perf_mode=dbl,
                    )
                nc.scalar.activation(
                    gbt[lnext % 2][:, b, :wid], gp[:], copyf,
                    scale=1.0 / W_SCALE,
                )
                store_own(lnext, b, wid)

            def layer(l, bias_t, mode):
                """One fused layer: phase A accumulates local (SBUF-direct)
                + piece-a chunks into PSUM and spills; phase B adds piece-b
                and runs the per-block tail. The next layer's piece-a AG
                fires from tail 3, piece-b at layer end."""
                w = l_wid[l]
                pw = HID if mode[0] != "final" else C_PAD
                # local pass first: pure-PE work with no DMA deps beyond the
                # previous layer's tails -- fills the AG/boundary window.
                # (Reuses the apsB PSUM buffers, which are idle here.)
                loc_acc = {}
                for b in range(MB):
                    psL = ppool.tile([128, pw], f32, tag="apsB", bufs=2)
                    local_mms(psL, b, l, w, True)
                    hl = wpool.tile([128, pw], bf16, tag="haccL", bufs=10)
                    nc.vector.tensor_copy(out=hl[:], in_=psL[:])
                    loc_acc[b] = hl
                tiles = {}
                for b in range(MB):
                    issue_block_calls(b, 0, full_p[0][l], w, tiles)
                haccs = {}
                for b in range(MB):
                    psA = ppool.tile([128, pw], f32, tag="apsA", bufs=2)
                    half_mms(psA, b, (0,), tiles, True, True)
                    # fold the local partial into the bf16 spill (same DVE op)
                    hc = wpool.tile([128, pw], bf16, tag="hacc", bufs=10)
                    nc.vector.tensor_add(out=hc[:], in0=psA[:], in1=loc_acc[b][:])
                    haccs[b] = hc
                for b in range(MB):
                    issue_block_calls(b, 1, full_p[1][l], w, tiles)
                for b in range(MB):
                    psB = ppool.tile([128, pw], f32, tag="apsB", bufs=2)
                    half_mms(psB, b, (1,), tiles, True, True)
                    block_tail(l, b, psB, haccs[b], bias_t, mode)
                    if mode[0] != "final" and b == 3:
                        allgather(own_p[0][mode[2]], full_p[0][mode[2]])
                if mode[0] != "final":
                    allgather(own_p[1][mode[2]], full_p[1][mode[2]])

            # ---- layers ----------------------------------------------------
            layer(0, b_sb[0], ("gemm", W2_sb, 1))
            layer(1, b_sb[1], ("gemm", W3_sb, 2))
            layer(2, b_sb[2], ("gemm", W4_sb, 3))
            layer(3, None, ("final",))

    nc.compile()
    return nc


# ----------------------------------------------------------------------------
# Entry point
# ----------------------------------------------------------------------------


def kernel(x, edge_index, batch, W1, b1, W2, b2, W3, b3, W4, b4, _trace=False):
    _install_drain_patch()
    from concourse.bass_utils import run_bass_kernel_spmd

    ka, in_maps = _prep_inputs(
        np.asarray(x, np.float32),
        np.asarray(edge_index),
        np.asarray(W1, np.float32), np.asarray(b1, np.float32),
        np.asarray(W2, np.float32), np.asarray(b2, np.float32),
        np.asarray(W3, np.float32), np.asarray(b3, np.float32),
        np.asarray(W4, np.float32), np.asarray(b4, np.float32),
    )
    key = tuple(ka)
    if key not in _cache:
        _cache[key] = _build(ka)
    nc = _cache[key]
    res = run_bass_kernel_spmd(
        nc, in_maps, core_ids=list(range(W_CORES)), trace=_trace
    )
    outp = np.concatenate(
        [res.results[c]["out"][:RPC] for c in range(W_CORES)], axis=0
    ).astype(np.float32)
    if _trace:
        return outp, res
    return outp


# revision 19
# speedup vs baseline: 1.0461x; 1.0221x over previous
"""GCN (4-layer, PyG-default GCNConv) forward on 8 Trainium2 NeuronCores.

Strategy (node-parallel / graph-parallel):
  - Nodes are partitioned contiguously across the 8 cores (1250 rows each,
    padded to 1280 = 10 blocks of 128).
  - Per layer: each core computes its row-slice of G = H @ W as a tiled
    fp8-DoubleRow PE GEMM. G is quantized to fp8e4m3 and kept both in SBUF
    (per-parity resident tile; serves all LOCAL edges directly) and
    AllGathered in TWO pieces (producer blocks 0-3 -> "a", 4-9 -> "b"):
    AG-a fires after block 3's store, AG-b at layer end, so each collective
    overlaps the neighbouring layer's aggregation. (ncfw executes
    collectives serially at ~20-50us wall each, so fewer is better.)
  - Aggregation (symmetric-normalized adjacency incl. self-loops):
      * LOCAL edges (same-core source, incl. self-loops): dense per
        (src-block, dst-block) fp8 S_loc matrices [128, 128] folded into
        DoubleRow pairs against the SBUF-resident G tile -- no DMA at all.
        These matmuls fill the AllGather entry-latency window at each layer
        boundary.
      * REMOTE edges: fp8 source rows fetched from the gathered pieces with
        dma_gather (chunked calls, single-packet) and summed on the PE as
        OUT_block += S_chunk.T @ MSG_chunk, with S a host-built fp8
        [128e, 128d] weight matrix; consecutive chunks are paired into fp8
        DoubleRow matmuls. Rows are deduped per (source, dst-block).
  - Layer 4 output G4 = H4 @ W4 is aggregated at fp8 256-wide (2 classes
    padded; 256B gather rows) and log_softmax is fused on-chip.
"""

import sys

sys.path.insert(0, "/opt/trn_rl_repo")

import numpy as np
import ml_dtypes

BF16 = ml_dtypes.bfloat16
F8 = ml_dtypes.float8_e4m3

# Problem constants (nn_GCN_39195871543847)
N, E, F_IN, HID, C = 10000, 160000, 2208, 512, 2
W_CORES = 8
RPC = N // W_CORES  # 1250 nodes per core
MB = 10  # 128-row blocks per core
RPAD = MB * 128  # 1280
# Two AllGather pieces over the producer's padded rows (ncfw runs
# collectives serially at ~20-50us wall each regardless of payload, so
# fewer AGs wins; the "a" piece fires mid-layer, "b" at layer end).
NPIECE = 2
PIECE_ROWS = (512, 768)
PIECE_BASE = (0, 512)
KFC = (F_IN + 127) // 128  # 18 contraction chunks for layer 1
KFP = KFC * 128  # 2304
C_PAD = 256  # pad 2 output classes to 256 fp8 (256B gather rows)
CALL = 2  # 128-idx chunks per dma_gather call (384 idx = 25 descs/engine;
# small enough that 2 calls fit in a queue's descriptor ring, so the pool
# engine can run one call ahead per queue instead of stalling on reclaim)
N_QUEUES = 4  # SWDGE queues for gather descriptor generation

W1_SCALE = 32.0  # fp8 e4m3 min normal is 2^-6; glorot W1 needs upscaling
W_SCALE = 16.0   # same for W2/W3/W4


def _install_drain_patch():
    """This container's walrus accepts at most one sync-wait per instruction;
    TileContext's final drain gets one wait per live semaphore. Split the
    extra waits onto single-wait NOPs."""
    import bass_rust
    import concourse.tile as tile
    from concourse.vector_clock import ScopedClock

    if getattr(tile.TileContext, "_drain_patch_installed", False):
        return

    def _drain_and_barrier(self, tick_clock, wait_clock):
        drain_inst = self.nc.sync.drain()
        wait_clock.add_sem_waits(
            drain_inst.ins, ScopedClock({None: tick_clock.global_clock})
        )
        si = drain_inst.ins.sync_info
        waits = list(si.on_wait or []) if si is not None else []
        if len(waits) > 1:
            si.on_wait = waits[:1]
            for w in waits[1:]:
                nop = self.nc.sync.nop(nofuse=True)
                nop.ins.sync_info = bass_rust.SyncInfo(on_wait=[w], on_update=[])
        self.nc.all_engine_barrier()
        assert self.sems is not None
        popped = self.nc._tile_sem_poison_stack.pop()
        assert popped is self._sem_poison
        self.nc.clear_and_free_semaphores(list(self.sems.allocated().values()))
        self.nc.all_engine_barrier()

    tile.TileContext._drain_and_barrier = _drain_and_barrier
    tile.TileContext._drain_patch_installed = True


# ----------------------------------------------------------------------------
# Host-side graph preprocessing
# ----------------------------------------------------------------------------


def _preprocess(edge_index):
    """Per core: dense local S stack (per src-block x dst-block pair, incl.
    self-loops) + per (128-dst block, remote piece) deduped gather slots with
    their S stack and gather indices.

    Remote pieces: 0 = producer rows 0-511 ("a"), 1 = rows 512-1279
    ("b"). Chunk stream order (shared tb layout): [a b0..b9][b b0..b9]
    """
    src = edge_index[0].astype(np.int64)
    dst = edge_index[1].astype(np.int64)
    loop = np.arange(N, dtype=np.int64)
    s = np.concatenate([src, loop])
    d = np.concatenate([dst, loop])
    deg = np.bincount(d, minlength=N).astype(np.float32)
    dinv = np.where(deg > 0, 1.0 / np.sqrt(deg), 0.0).astype(np.float32)
    norm = (dinv[s] * dinv[d]).astype(np.float64)

    core = d // RPC
    slot_rows = {}
    edge_tuples = {}
    ka = np.zeros((MB, NPIECE), np.int64)
    sloc_np = []
    for c in range(W_CORES):
        m = core == c
        sc, dc, wc = s[m], d[m] - c * RPC, norm[m]
        s_core = sc // RPC
        s_loc = sc % RPC
        local = s_core == c

        # dense local S: [128(k=src pos), 100 (b*10+m), 128 (dst pos)]
        S_loc = np.zeros((128, MB * MB, 128), np.float32)
        lm = s_loc[local] // 128
        lk = s_loc[local] % 128
        lb = dc[local] // 128
        lj = dc[local] % 128
        np.add.at(S_loc, (lk, lb * MB + lm, lj), wc[local])
        sloc_np.append(S_loc.astype(F8))

        # remote classes
        rm = ~local
        scr, dcr, wcr = sc[rm], dc[rm], wc[rm]
        s_corer = scr // RPC
        s_locr = scr % RPC
        piece = np.where(s_locr < 512, 0, 1)
        g_row = np.zeros_like(s_locr)
        for h in range(NPIECE):
            mm = piece == h
            g_row[mm] = s_corer[mm] * PIECE_ROWS[h] + (s_locr[mm] - PIECE_BASE[h])
        blk = dcr // 128
        mloc = dcr % 128
        for b in range(MB):
            for h in range(NPIECE):
                mm = (blk == b) & (piece == h)
                rows = g_row[mm]
                ml = mloc[mm]
                ww = wcr[mm]
                uniq, inv = np.unique(rows, return_inverse=True)
                slot_rows[(c, b, h)] = uniq
                edge_tuples[(c, b, h)] = (inv, ml, ww)
                ka[b, h] = max(ka[b, h], 1, (len(uniq) + 127) // 128)

    seg_order = [(h, b) for h in range(NPIECE) for b in range(MB)]
    tb_off = {}
    t = 0
    for h, b in seg_order:
        tb_off[(b, h)] = t
        t += int(ka[b, h])
    T = t

    s_list, idx_list = [], []
    for c in range(W_CORES):
        S = np.zeros((T, 128, 128), np.float32)
        idx_flat = np.zeros(T * 128, np.int16)
        for h, b in seg_order:
            if (c, b, h) not in slot_rows:
                continue
            t0 = tb_off[(b, h)]
            uniq = slot_rows[(c, b, h)]
            inv, ml, ww = edge_tuples[(c, b, h)]
            k = np.arange(len(uniq))
            tt = t0 + k // 128
            kk = k % 128
            idx_flat[tt * 128 + kk] = uniq.astype(np.int16)
            np.add.at(S, (tt[inv], kk[inv], ml), ww)
        lay16 = idx_flat.reshape(T * 8, 16).T  # [16, T*8]
        idx_list.append(np.tile(lay16, (8, 1)).astype(np.int16))
        # SBUF-resident layout [128 partitions(k), T, 128(m)]
        s_list.append(
            np.ascontiguousarray(S.transpose(1, 0, 2)).astype(F8)
        )
    ka_t = tuple(
        (int(ka[b, 0]), int(ka[b, 1])) for b in range(MB)
    )
    return ka_t, s_list, idx_list, sloc_np


def _prep_inputs(x, edge_index, W1, b1, W2, b2, W3, b3, W4, b4):
    ka, s_list, idx_list, sloc_list = _preprocess(edge_index)

    # xT per core: [MB, 128(p), KFC, 128(j)]; xT[m,p,k,j] = x[c*RPC+m*128+j, k*128+p]
    xts = []
    for c in range(W_CORES):
        xp = np.zeros((RPAD, KFP), np.float32)
        xp[:RPC, :F_IN] = x[c * RPC : (c + 1) * RPC]
        xt = xp.reshape(MB, 128, KFC, 128).transpose(0, 3, 2, 1)
        xts.append(np.ascontiguousarray(xt).astype(F8))

    W1p = np.zeros((KFP, HID), np.float32)
    W1p[:F_IN] = W1 * W1_SCALE
    W1l = np.ascontiguousarray(
        W1p.reshape(KFC, 128, HID).transpose(1, 0, 2)
    ).astype(F8)
    W2l = np.ascontiguousarray(
        (W2 * W_SCALE).reshape(4, 128, HID).transpose(1, 0, 2)
    ).astype(F8)
    W3l = np.ascontiguousarray(
        (W3 * W_SCALE).reshape(4, 128, HID).transpose(1, 0, 2)
    ).astype(F8)
    W4p = np.zeros((HID, C_PAD), np.float32)
    W4p[:, :C] = W4 * W_SCALE
    W4l = np.ascontiguousarray(
        W4p.reshape(4, 128, C_PAD).transpose(1, 0, 2)
    ).astype(F8)

    b1r = np.broadcast_to(b1, (128, HID)).astype(np.float32).copy()
    b2r = np.broadcast_to(b2, (128, HID)).astype(np.float32).copy()
    b3r = np.broadcast_to(b3, (128, HID)).astype(np.float32).copy()
    b4r = np.zeros((128, C_PAD), np.float32)
    b4r[:, :C] = b4

    in_maps = []
    for c in range(W_CORES):
        in_maps.append(
            {
                "xT": xts[c],
                "W1l": W1l, "W2l": W2l, "W3l": W3l, "W4l": W4l,
                "b1r": b1r, "b2r": b2r, "b3r": b3r, "b4r": b4r,
                "S_in": s_list[c],
                "Sloc_in": sloc_list[c],
                "idx_in": idx_list[c],
            }
        )
    return ka, in_maps


# ----------------------------------------------------------------------------
# Bass kernel builder
# ----------------------------------------------------------------------------

_cache = {}


def _build(ka):
    import concourse.bass as bass
    import concourse.mybir as mybir
    from concourse.bacc import Bacc
    from concourse.tile import TileContext
    from concourse.masks import make_identity

    f32 = mybir.dt.float32
    bf16 = mybir.dt.bfloat16
    f8 = mybir.dt.float8e4
    i16 = mybir.dt.int16

    # chunk stream layout (must match _preprocess seg_order)
    seg_order = [(h, b) for h in range(NPIECE) for b in range(MB)]
    tb_off = {}
    t = 0
    for h, b in seg_order:
        tb_off[(b, h)] = t
        t += ka[b][h]
    T = t

    nc = Bacc(num_devices=W_CORES, num_swdge_queues=N_QUEUES)
    gq = [0]  # round-robin cursor over gather queues

    xT = nc.dram_tensor("xT", [MB, 128, KFC, 128], f8, kind="ExternalInput")
    W1l = nc.dram_tensor("W1l", [128, KFC, HID], f8, kind="ExternalInput")
    W2l = nc.dram_tensor("W2l", [128, 4, HID], f8, kind="ExternalInput")
    W3l = nc.dram_tensor("W3l", [128, 4, HID], f8, kind="ExternalInput")
    W4l = nc.dram_tensor("W4l", [128, 4, C_PAD], f8, kind="ExternalInput")
    b1r = nc.dram_tensor("b1r", [128, HID], f32, kind="ExternalInput")
    b2r = nc.dram_tensor("b2r", [128, HID], f32, kind="ExternalInput")
    b3r = nc.dram_tensor("b3r", [128, HID], f32, kind="ExternalInput")
    b4r = nc.dram_tensor("b4r", [128, C_PAD], f32, kind="ExternalInput")
    S_in = nc.dram_tensor("S_in", [128, T, 128], f8, kind="ExternalInput")
    Sloc_in = nc.dram_tensor(
        "Sloc_in", [128, MB * MB, 128], f8, kind="ExternalInput"
    )
    idx_in = nc.dram_tensor("idx_in", [128, T * 8], i16, kind="ExternalInput")
    out = nc.dram_tensor("out", [RPAD, C], f32, kind="ExternalOutput")

    # per-layer bounce pieces + gathered pieces (all fp8)
    l_wid = [HID, HID, HID, C_PAD]
    own_p = [[] for _ in range(NPIECE)]
    full_p = [[] for _ in range(NPIECE)]
    for l in range(4):
        for h in range(NPIECE):
            own_p[h].append(
                nc.dram_tensor(
                    f"own{'abc'[h]}{l}", [PIECE_ROWS[h], l_wid[l]], f8,
                    kind="Internal",
                )
            )
            full_p[h].append(
                nc.dram_tensor(
                    f"full{'abc'[h]}{l}",
                    [W_CORES * PIECE_ROWS[h], l_wid[l]], f8,
                    kind="Internal", addr_space="Shared",
                )
            )

    rg = [list(range(W_CORES))]

    with TileContext(nc) as tc:
        with (
            tc.tile_pool(name="const", bufs=1) as cpool,
            tc.tile_pool(name="work", bufs=2) as wpool,
            tc.tile_pool(name="psum", bufs=2, space="PSUM") as ppool,
        ):
            relu = mybir.ActivationFunctionType.Relu
            copyf = mybir.ActivationFunctionType.Copy
            dbl = mybir.MatmulPerfMode.DoubleRow

            # index-count registers for the call windows (per block-class)
            sizes = {CALL * 128}
            for b in range(MB):
                for h in range(NPIECE):
                    if ka[b][h] % CALL:
                        sizes.add((ka[b][h] % CALL) * 128)
            r_cnt = {n: nc.gpsimd.to_reg(n) for n in sizes}

            # ---- layer-1 GEMM inputs first (critical path) -------------------
            W1_sb = []
            for k0 in range(0, KFC, 6):
                k1 = min(k0 + 6, KFC)
                wg = cpool.tile([128, k1 - k0, HID], f8, tag=f"w1g{k0}")
                nc.sync.dma_start(out=wg[:], in_=W1l[:, k0:k1, :])
                W1_sb.append(wg)

            # resident G tiles, one per parity: [128, MB, HID] fp8.
            # Layer l's GEMM writes gbt[l%2]; layer l's local aggregation
            # reads it. (Layer-3 G4 occupies [:, :, :C_PAD].)
            gbt = [
                cpool.tile(
                    [128, MB, HID], f8, tag=f"gbt{p}", name=f"gbt{p}"
                )
                for p in range(2)
            ]

            def allgather(own, full):
                nc.gpsimd.collective_compute(
                    "AllGather",
                    mybir.AluOpType.bypass,
                    ins=[own[:]],
                    outs=[full[:]],
                    replica_groups=rg,
                )

            def store_own(lslot, m, wid):
                """DMA gbt slice for block m to its own piece rows."""
                h = 0 if m < 4 else 1
                r0 = m * 128 - PIECE_BASE[h]
                nc.sync.dma_start(
                    out=own_p[h][lslot][r0 : r0 + 128, :],
                    in_=gbt[lslot % 2][:, m, :wid],
                )

            def gemm_l1():
                # fp8 DoubleRow: 9 k-pair matmuls per block.
                for m in range(MB):
                    xm = wpool.tile([128, KFC, 128], f8, tag="xm", bufs=3)
                    nc.sync.dma_start(out=xm[:], in_=xT[m])
                    ps = ppool.tile([128, HID], f32, tag="gps")
                    for k in range(0, KFC, 2):
                        nc.tensor.matmul(
                            ps[:],
                            lhsT=xm[:, k : k + 2, :],
                            rhs=W1_sb[k // 6][:, k % 6 : k % 6 + 2, :],
                            start=(k == 0),
                            stop=(k == KFC - 2),
                            perf_mode=dbl,
                        )
                    nc.scalar.activation(
                        gbt[0][:, m, :], ps[:], copyf, scale=1.0 / W1_SCALE
                    )
                    store_own(0, m, HID)
                    if m == 3:
                        allgather(own_p[0][0], full_p[0][0])
                    elif m == MB - 1:
                        allgather(own_p[1][0], full_p[1][0])

            gemm_l1()

            # ---- remaining resident tensors (overlap the first collective) ---
            idx_sb = cpool.tile([128, T * 8], i16)
            nc.sync.dma_start(out=idx_sb[:], in_=idx_in[:])
            Sloc_sb = cpool.tile([128, MB * MB, 128], f8)
            nc.sync.dma_start(out=Sloc_sb[:], in_=Sloc_in[:])
            S_sb = cpool.tile([128, T, 128], f8)
            nc.sync.dma_start(out=S_sb[:], in_=S_in[:])
            W2_sb = cpool.tile([128, 4, HID], f8)
            nc.sync.dma_start(out=W2_sb[:], in_=W2l[:])
            W3_sb = cpool.tile([128, 4, HID], f8)
            nc.sync.dma_start(out=W3_sb[:], in_=W3l[:])
            W4_sb = cpool.tile([128, 4, C_PAD], f8)
            nc.sync.dma_start(out=W4_sb[:], in_=W4l[:])
            b_sb = []
            for nm, srcb in (("b1", b1r), ("b2", b2r), ("b3", b3r)):
                tle = cpool.tile([128, HID], f32, tag=f"bias_{nm}")
                nc.sync.dma_start(out=tle[:], in_=srcb[:])
                b_sb.append(tle)
            b4_sb = cpool.tile([128, C_PAD], f32)
            nc.sync.dma_start(out=b4_sb[:], in_=b4r[:])
            id_bf = cpool.tile([128, 128], bf16)
            make_identity(nc, id_bf[:])

            def issue_block_calls(b, h, fsrc, w, tiles):
                """Issue the gather calls for block b's piece-h chunk run;
                record tiles keyed by window-start chunk index."""
                t0 = tb_off[(b, h)]
                t1 = t0 + ka[b][h]
                for tw in range(t0, t1, CALL):
                    nk = min(CALL, t1 - tw)
                    msg = wpool.tile([128, CALL, w], f8, tag=f"msg{w}", bufs=24)
                    nc.gpsimd.dma_gather(
                        out_ap=msg[:, :nk, :],
                        in_ap=fsrc[:],
                        idxs_ap=idx_sb[:, tw * 8 : (tw + nk) * 8],
                        num_idxs=nk * 128,
                        num_idxs_reg=r_cnt[nk * 128],
                        elem_size=w,
                        queue_num=gq[0],
                    )
                    gq[0] = (gq[0] + 1) % N_QUEUES
                    tiles[tw] = msg

            def local_mms(ps, b, l, w, start):
                """Dense local aggregation for dst block b: 5 DoubleRow
                matmuls against the parity-resident G tile."""
                par = l % 2
                for m in range(0, MB, 2):
                    nc.tensor.matmul(
                        ps[:],
                        lhsT=Sloc_sb[:, b * MB + m : b * MB + m + 2, :],
                        rhs=gbt[par][:, m : m + 2, :w],
                        start=(start and m == 0),
                        stop=False,
                        perf_mode=dbl,
                    )

            def half_mms(ps, b, hs, tiles, start, stop):
                """Accumulate block b's chunks for the pieces in hs into ps,
                pairing consecutive chunks within a call window (DoubleRow)."""
                runs = []  # (t, tw, pair)
                for h in hs:
                    t0 = tb_off[(b, h)]
                    t1 = t0 + ka[b][h]
                    for tw in range(t0, t1, CALL):
                        nk = min(CALL, t1 - tw)
                        i = 0
                        while i < nk:
                            if i + 1 < nk:
                                runs.append((tw + i, tw, True))
                                i += 2
                            else:
                                runs.append((tw + i, tw, False))
                                i += 1
                for i, (t, tw, pair) in enumerate(runs):
                    msg = tiles[tw]
                    if pair:
                        nc.tensor.matmul(
                            ps[:],
                            lhsT=S_sb[:, t : t + 2, :],
                            rhs=msg[:, t - tw : t - tw + 2, :],
                            start=(start and i == 0),
                            stop=(stop and i == len(runs) - 1),
                            perf_mode=dbl,
                        )
                    else:
                        nc.tensor.matmul(
                            ps[:],
                            lhsT=S_sb[:, t, :],
                            rhs=msg[:, t - tw, :],
                            start=(start and i == 0),
                            stop=(stop and i == len(runs) - 1),
                        )

            lsm_t2 = {}

            def logsoftmax_block(ps, hacc, m):
                """Vector-only prefix of log_softmax for one block; the
                Exp/Ln scalar passes are batched afterwards so the activation
                table is loaded twice total instead of twice per block."""
                lg0 = wpool.tile([128, C_PAD], f32, tag="lg0")
                nc.vector.tensor_add(out=lg0[:], in0=ps[:], in1=hacc[:])
                lg = wpool.tile([128, C_PAD], f32, tag="lg")
                nc.vector.tensor_add(out=lg[:], in0=lg0[:], in1=b4_sb[:])
                mx = wpool.tile([128, 1], f32, tag="mx")
                nc.vector.tensor_reduce(
                    out=mx[:], in_=lg[:, :C], axis=mybir.AxisListType.X,
                    op=mybir.AluOpType.max,
                )
                t2 = cpool.tile([128, C], f32, tag=f"t2_{m}", name=f"t2_{m}")
                nc.vector.tensor_scalar(
                    out=t2[:], in0=lg[:, :C], scalar1=mx[:], scalar2=None,
                    op0=mybir.AluOpType.subtract,
                )
                lsm_t2[m] = t2

            def logsoftmax_finish():
                e2s = {}
                for m in range(MB):
                    e2 = wpool.tile([128, C], f32, tag="e2", bufs=10)
                    nc.scalar.activation(
                        e2[:], lsm_t2[m][:], mybir.ActivationFunctionType.Exp
                    )
                    e2s[m] = e2
                sms = {}
                for m in range(MB):
                    sm = wpool.tile([128, 1], f32, tag="sm", bufs=10)
                    nc.vector.tensor_reduce(
                        out=sm[:], in_=e2s[m][:], axis=mybir.AxisListType.X,
                        op=mybir.AluOpType.add,
                    )
                    sms[m] = sm
                lss = {}
                for m in range(MB):
                    ls = wpool.tile([128, 1], f32, tag="ls", bufs=10)
                    nc.scalar.activation(
                        ls[:], sms[m][:], mybir.ActivationFunctionType.Ln
                    )
                    lss[m] = ls
                for m in range(MB):
                    o2 = wpool.tile([128, C], f32, tag="o2", bufs=4)
                    nc.vector.tensor_scalar(
                        out=o2[:], in0=lsm_t2[m][:], scalar1=lss[m][:],
                        scalar2=None, op0=mybir.AluOpType.subtract,
                    )
                    nc.sync.dma_start(
                        out=out[m * 128 : (m + 1) * 128, :], in_=o2[:]
                    )

            def block_tail(l, b, ps, hacc, bias_t, mode):
                """Bias + phase-A partial + relu + transpose + next-layer GEMM
                + store for one finished block (or the final classifier)."""
                if mode[0] == "final":
                    logsoftmax_block(ps, hacc, b)
                    return
                h0 = wpool.tile([128, HID], f32, tag="h0", bufs=3)
                nc.vector.tensor_add(out=h0[:], in0=ps[:], in1=hacc[:])
                hf = wpool.tile([128, HID], f32, tag="hf", bufs=3)
                nc.vector.tensor_add(out=hf[:], in0=h0[:], in1=bias_t[:])
                hb = wpool.tile([128, HID], bf16, tag="hb", bufs=3)
                nc.scalar.activation(hb[:], hf[:], relu)
                ht = wpool.tile([128, 4, 128], f8, tag="ht", bufs=4)
                for gg in range(4):
                    tp = ppool.tile([128, 128], bf16, tag="tps", bufs=1)
                    nc.tensor.transpose(
                        tp[:], hb[:, gg * 128 : (gg + 1) * 128], id_bf[:]
                    )
                    nc.vector.tensor_copy(out=ht[:, gg, :], in_=tp[:])
                _, w_sb, lnext = mode
                wid = l_wid[lnext]
                gp = ppool.tile([128, wid], f32, tag="gps")
                for k in range(0, 4, 2):
                    nc.tensor.matmul(
                        gp[:],
                        lhsT=ht[:, k : k + 2, :],
                        rhs=w_sb[:, k : k + 2, :],
                        start=(k == 0),
                        stop=(k == 2),
                        perf_mode=dbl,
                    )
                nc.scalar.activation(
                    gbt[lnext % 2][:, b, :wid], gp[:], copyf,
                    scale=1.0 / W_SCALE,
                )
                store_own(lnext, b, wid)

            def layer(l, bias_t, mode):
                """One fused layer: phase A accumulates local (SBUF-direct)
                + piece-a chunks into PSUM and spills; phase B adds piece-b
                and runs the per-block tail. The next layer's piece-a AG
                fires from tail 3, piece-b at layer end."""
                w = l_wid[l]
                pw = HID if mode[0] != "final" else C_PAD
                tiles = {}
                for b in range(MB):
                    issue_block_calls(b, 0, full_p[0][l], w, tiles)
                haccs = {}
                for b in range(MB):
                    psA = ppool.tile([128, pw], f32, tag="apsA", bufs=2)
                    local_mms(psA, b, l, w, True)
                    half_mms(psA, b, (0,), tiles, False, True)
                    # bf16 spill on DVE (2x rate, off the scalar engine)
                    hc = wpool.tile([128, pw], bf16, tag="hacc", bufs=10)
                    nc.vector.tensor_copy(out=hc[:], in_=psA[:])
                    haccs[b] = hc
                for b in range(MB):
                    issue_block_calls(b, 1, full_p[1][l], w, tiles)
                for b in range(MB):
                    psB = ppool.tile([128, pw], f32, tag="apsB", bufs=2)
                    half_mms(psB, b, (1,), tiles, True, True)
                    block_tail(l, b, psB, haccs[b], bias_t, mode)
                    if mode[0] != "final" and b == 3:
                        allgather(own_p[0][mode[2]], full_p[0][mode[2]])
                if mode[0] != "final":
                    allgather(own_p[1][mode[2]], full_p[1][mode[2]])
                else:
                    logsoftmax_finish()

            # ---- layers ----------------------------------------------------
            layer(0, b_sb[0], ("gemm", W2_sb, 1))
            layer(1, b_sb[1], ("gemm", W3_sb, 2))
            layer(2, b_sb[2], ("gemm", W4_sb, 3))
            layer(3, None, ("final",))

    nc.compile()
    return nc


# ----------------------------------------------------------------------------
# Entry point
# ----------------------------------------------------------------------------


def kernel(x, edge_index, batch, W1, b1, W2, b2, W3, b3, W4, b4, _trace=False):
    _install_drain_patch()
    from concourse.bass_utils import run_bass_kernel_spmd

    ka, in_maps = _prep_inputs(
        np.asarray(x, np.float32),
        np.asarray(edge_index),
        np.asarray(W1, np.float32), np.asarray(b1, np.float32),
        np.asarray(W2, np.float32), np.asarray(b2, np.float32),
        np.asarray(W3, np.float32), np.asarray(b3, np.float32),
        np.asarray(W4, np.float32), np.asarray(b4, np.float32),
    )
    key = tuple(ka)
    if key not in _cache:
        _cache[key] = _build(ka)
    nc = _cache[key]
    res = run_bass_kernel_spmd(
        nc, in_maps, core_ids=list(range(W_CORES)), trace=_trace
    )
    outp = np.concatenate(
        [res.results[c]["out"][:RPC] for c in range(W_CORES)], axis=0
    ).astype(np.float32)
    if _trace:
        return outp, res
    return outp


# revision 21
# speedup vs baseline: 1.0521x; 1.0057x over previous
"""GCN (4-layer, PyG-default GCNConv) forward on 8 Trainium2 NeuronCores.

Strategy (node-parallel / graph-parallel):
  - Nodes are partitioned contiguously across the 8 cores (1250 rows each,
    padded to 1280 = 10 blocks of 128).
  - Per layer: each core computes its row-slice of G = H @ W as a tiled
    fp8-DoubleRow PE GEMM. G is quantized to fp8e4m3 and kept both in SBUF
    (per-parity resident tile; serves all LOCAL edges directly) and
    AllGathered in TWO pieces (producer blocks 0-3 -> "a", 4-9 -> "b"):
    AG-a fires after block 3's store, AG-b at layer end, so each collective
    overlaps the neighbouring layer's aggregation. (ncfw executes
    collectives serially at ~20-50us wall each, so fewer is better.)
  - Aggregation (symmetric-normalized adjacency incl. self-loops):
      * LOCAL edges (same-core source, incl. self-loops): dense per
        (src-block, dst-block) fp8 S_loc matrices [128, 128] folded into
        DoubleRow pairs against the SBUF-resident G tile -- no DMA at all.
        These matmuls fill the AllGather entry-latency window at each layer
        boundary.
      * REMOTE edges: fp8 source rows fetched from the gathered pieces with
        dma_gather (chunked calls, single-packet) and summed on the PE as
        OUT_block += S_chunk.T @ MSG_chunk, with S a host-built fp8
        [128e, 128d] weight matrix; consecutive chunks are paired into fp8
        DoubleRow matmuls. Rows are deduped per (source, dst-block).
  - Layer 4 output G4 = H4 @ W4 is aggregated at fp8 256-wide (2 classes
    padded; 256B gather rows) and log_softmax is fused on-chip.
"""

import sys

sys.path.insert(0, "/opt/trn_rl_repo")

import numpy as np
import ml_dtypes

BF16 = ml_dtypes.bfloat16
F8 = ml_dtypes.float8_e4m3

# Problem constants (nn_GCN_39195871543847)
N, E, F_IN, HID, C = 10000, 160000, 2208, 512, 2
W_CORES = 8
RPC = N // W_CORES  # 1250 nodes per core
MB = 10  # 128-row blocks per core
RPAD = MB * 128  # 1280
# Two AllGather pieces over the producer's padded rows (ncfw runs
# collectives serially at ~20-50us wall each regardless of payload, so
# fewer AGs wins; the "a" piece fires mid-layer, "b" at layer end).
NPIECE = 2
PIECE_ROWS = (512, 768)
PIECE_BASE = (0, 512)
KFC = (F_IN + 127) // 128  # 18 contraction chunks for layer 1
KFP = KFC * 128  # 2304
C_PAD = 256  # pad 2 output classes to 256 fp8 (256B gather rows)
CALL = 2  # 128-idx chunks per dma_gather call (384 idx = 25 descs/engine;
# small enough that 2 calls fit in a queue's descriptor ring, so the pool
# engine can run one call ahead per queue instead of stalling on reclaim)
N_QUEUES = 4  # SWDGE queues for gather descriptor generation

W1_SCALE = 32.0  # fp8 e4m3 min normal is 2^-6; glorot W1 needs upscaling
W_SCALE = 16.0   # same for W2/W3/W4


def _install_drain_patch():
    """This container's walrus accepts at most one sync-wait per instruction;
    TileContext's final drain gets one wait per live semaphore. Split the
    extra waits onto single-wait NOPs."""
    import bass_rust
    import concourse.tile as tile
    from concourse.vector_clock import ScopedClock

    if getattr(tile.TileContext, "_drain_patch_installed", False):
        return

    def _drain_and_barrier(self, tick_clock, wait_clock):
        drain_inst = self.nc.sync.drain()
        wait_clock.add_sem_waits(
            drain_inst.ins, ScopedClock({None: tick_clock.global_clock})
        )
        si = drain_inst.ins.sync_info
        waits = list(si.on_wait or []) if si is not None else []
        if len(waits) > 1:
            si.on_wait = waits[:1]
            for w in waits[1:]:
                nop = self.nc.sync.nop(nofuse=True)
                nop.ins.sync_info = bass_rust.SyncInfo(on_wait=[w], on_update=[])
        self.nc.all_engine_barrier()
        assert self.sems is not None
        popped = self.nc._tile_sem_poison_stack.pop()
        assert popped is self._sem_poison
        self.nc.clear_and_free_semaphores(list(self.sems.allocated().values()))
        self.nc.all_engine_barrier()

    tile.TileContext._drain_and_barrier = _drain_and_barrier
    tile.TileContext._drain_patch_installed = True

    # Activation-table consolidation: the table chooser picks the FIRST
    # act_func_set containing each function, which puts Exp and Ln in
    # different sets and thrashes ACT_TABLE_LOAD (1.3us each) between them.
    # natural_log_exp_and_others contains exp+ln+relu+copy together; strip
    # those functions from the sets that precede it (keeping dict order, so
    # act_func_set_id indices stay valid) to route everything there.
    import functools
    import concourse.hw_specs as hw_specs
    import concourse.bacc as bacc_mod
    from concourse import mybir as _mb

    orig_gat = hw_specs.get_activation_tables

    @functools.cache
    def _merged_tables(arch):
        t = {k: set(v) for k, v in orig_gat(arch).items()}
        key = "natural_log_exp_and_others"
        if key in t:
            ours = {
                _mb.ActivationFunctionType.Exp,
                _mb.ActivationFunctionType.Ln,
                _mb.ActivationFunctionType.Relu,
                _mb.ActivationFunctionType.Copy,
                _mb.ActivationFunctionType.Identity,
            }
            assert ours <= t[key]
            for k in t:
                if k != key:
                    t[k] = t[k] - ours
        return t

    hw_specs.get_activation_tables = _merged_tables
    bacc_mod.get_activation_tables = _merged_tables


# ----------------------------------------------------------------------------
# Host-side graph preprocessing
# ----------------------------------------------------------------------------


def _preprocess(edge_index):
    """Per core: dense local S stack (per src-block x dst-block pair, incl.
    self-loops) + per (128-dst block, remote piece) deduped gather slots with
    their S stack and gather indices.

    Remote pieces: 0 = producer rows 0-511 ("a"), 1 = rows 512-1279
    ("b"). Chunk stream order (shared tb layout): [a b0..b9][b b0..b9]
    """
    src = edge_index[0].astype(np.int64)
    dst = edge_index[1].astype(np.int64)
    loop = np.arange(N, dtype=np.int64)
    s = np.concatenate([src, loop])
    d = np.concatenate([dst, loop])
    deg = np.bincount(d, minlength=N).astype(np.float32)
    dinv = np.where(deg > 0, 1.0 / np.sqrt(deg), 0.0).astype(np.float32)
    norm = (dinv[s] * dinv[d]).astype(np.float64)

    core = d // RPC
    slot_rows = {}
    edge_tuples = {}
    ka = np.zeros((MB, NPIECE), np.int64)
    sloc_np = []
    for c in range(W_CORES):
        m = core == c
        sc, dc, wc = s[m], d[m] - c * RPC, norm[m]
        s_core = sc // RPC
        s_loc = sc % RPC
        local = s_core == c

        # dense local S: [128(k=src pos), 100 (b*10+m), 128 (dst pos)]
        S_loc = np.zeros((128, MB * MB, 128), np.float32)
        lm = s_loc[local] // 128
        lk = s_loc[local] % 128
        lb = dc[local] // 128
        lj = dc[local] % 128
        np.add.at(S_loc, (lk, lb * MB + lm, lj), wc[local])
        sloc_np.append(S_loc.astype(F8))

        # remote classes
        rm = ~local
        scr, dcr, wcr = sc[rm], dc[rm], wc[rm]
        s_corer = scr // RPC
        s_locr = scr % RPC
        piece = np.where(s_locr < 512, 0, 1)
        g_row = np.zeros_like(s_locr)
        for h in range(NPIECE):
            mm = piece == h
            g_row[mm] = s_corer[mm] * PIECE_ROWS[h] + (s_locr[mm] - PIECE_BASE[h])
        blk = dcr // 128
        mloc = dcr % 128
        for b in range(MB):
            for h in range(NPIECE):
                mm = (blk == b) & (piece == h)
                rows = g_row[mm]
                ml = mloc[mm]
                ww = wcr[mm]
                uniq, inv = np.unique(rows, return_inverse=True)
                slot_rows[(c, b, h)] = uniq
                edge_tuples[(c, b, h)] = (inv, ml, ww)
                ka[b, h] = max(ka[b, h], 1, (len(uniq) + 127) // 128)

    seg_order = [(h, b) for h in range(NPIECE) for b in range(MB)]
    tb_off = {}
    t = 0
    for h, b in seg_order:
        tb_off[(b, h)] = t
        t += int(ka[b, h])
    T = t

    s_list, idx_list = [], []
    for c in range(W_CORES):
        S = np.zeros((T, 128, 128), np.float32)
        idx_flat = np.zeros(T * 128, np.int16)
        for h, b in seg_order:
            if (c, b, h) not in slot_rows:
                continue
            t0 = tb_off[(b, h)]
            uniq = slot_rows[(c, b, h)]
            inv, ml, ww = edge_tuples[(c, b, h)]
            k = np.arange(len(uniq))
            tt = t0 + k // 128
            kk = k % 128
            idx_flat[tt * 128 + kk] = uniq.astype(np.int16)
            np.add.at(S, (tt[inv], kk[inv], ml), ww)
        lay16 = idx_flat.reshape(T * 8, 16).T  # [16, T*8]
        idx_list.append(np.tile(lay16, (8, 1)).astype(np.int16))
        # SBUF-resident layout [128 partitions(k), T, 128(m)]
        s_list.append(
            np.ascontiguousarray(S.transpose(1, 0, 2)).astype(F8)
        )
    ka_t = tuple(
        (int(ka[b, 0]), int(ka[b, 1])) for b in range(MB)
    )
    return ka_t, s_list, idx_list, sloc_np


def _prep_inputs(x, edge_index, W1, b1, W2, b2, W3, b3, W4, b4):
    ka, s_list, idx_list, sloc_list = _preprocess(edge_index)

    # xT per core: [MB, 128(p), KFC, 128(j)]; xT[m,p,k,j] = x[c*RPC+m*128+j, k*128+p]
    xts = []
    for c in range(W_CORES):
        xp = np.zeros((RPAD, KFP), np.float32)
        xp[:RPC, :F_IN] = x[c * RPC : (c + 1) * RPC]
        xt = xp.reshape(MB, 128, KFC, 128).transpose(0, 3, 2, 1)
        xts.append(np.ascontiguousarray(xt).astype(F8))

    W1p = np.zeros((KFP, HID), np.float32)
    W1p[:F_IN] = W1 * W1_SCALE
    W1l = np.ascontiguousarray(
        W1p.reshape(KFC, 128, HID).transpose(1, 0, 2)
    ).astype(F8)
    W2l = np.ascontiguousarray(
        (W2 * W_SCALE).reshape(4, 128, HID).transpose(1, 0, 2)
    ).astype(F8)
    W3l = np.ascontiguousarray(
        (W3 * W_SCALE).reshape(4, 128, HID).transpose(1, 0, 2)
    ).astype(F8)
    W4p = np.zeros((HID, C_PAD), np.float32)
    W4p[:, :C] = W4 * W_SCALE
    W4l = np.ascontiguousarray(
        W4p.reshape(4, 128, C_PAD).transpose(1, 0, 2)
    ).astype(F8)

    b1r = np.broadcast_to(b1, (128, HID)).astype(np.float32).copy()
    b2r = np.broadcast_to(b2, (128, HID)).astype(np.float32).copy()
    b3r = np.broadcast_to(b3, (128, HID)).astype(np.float32).copy()
    b4r = np.zeros((128, C_PAD), np.float32)
    b4r[:, :C] = b4

    in_maps = []
    for c in range(W_CORES):
        in_maps.append(
            {
                "xT": xts[c],
                "W1l": W1l, "W2l": W2l, "W3l": W3l, "W4l": W4l,
                "b1r": b1r, "b2r": b2r, "b3r": b3r, "b4r": b4r,
                "S_in": s_list[c],
                "Sloc_in": sloc_list[c],
                "idx_in": idx_list[c],
            }
        )
    return ka, in_maps


# ----------------------------------------------------------------------------
# Bass kernel builder
# ----------------------------------------------------------------------------

_cache = {}


def _build(ka):
    import concourse.bass as bass
    import concourse.mybir as mybir
    from concourse.bacc import Bacc
    from concourse.tile import TileContext
    from concourse.masks import make_identity

    f32 = mybir.dt.float32
    bf16 = mybir.dt.bfloat16
    f8 = mybir.dt.float8e4
    i16 = mybir.dt.int16

    # chunk stream layout (must match _preprocess seg_order)
    seg_order = [(h, b) for h in range(NPIECE) for b in range(MB)]
    tb_off = {}
    t = 0
    for h, b in seg_order:
        tb_off[(b, h)] = t
        t += ka[b][h]
    T = t

    nc = Bacc(num_devices=W_CORES, num_swdge_queues=N_QUEUES)
    gq = [0]  # round-robin cursor over gather queues

    xT = nc.dram_tensor("xT", [MB, 128, KFC, 128], f8, kind="ExternalInput")
    W1l = nc.dram_tensor("W1l", [128, KFC, HID], f8, kind="ExternalInput")
    W2l = nc.dram_tensor("W2l", [128, 4, HID], f8, kind="ExternalInput")
    W3l = nc.dram_tensor("W3l", [128, 4, HID], f8, kind="ExternalInput")
    W4l = nc.dram_tensor("W4l", [128, 4, C_PAD], f8, kind="ExternalInput")
    b1r = nc.dram_tensor("b1r", [128, HID], f32, kind="ExternalInput")
    b2r = nc.dram_tensor("b2r", [128, HID], f32, kind="ExternalInput")
    b3r = nc.dram_tensor("b3r", [128, HID], f32, kind="ExternalInput")
    b4r = nc.dram_tensor("b4r", [128, C_PAD], f32, kind="ExternalInput")
    S_in = nc.dram_tensor("S_in", [128, T, 128], f8, kind="ExternalInput")
    Sloc_in = nc.dram_tensor(
        "Sloc_in", [128, MB * MB, 128], f8, kind="ExternalInput"
    )
    idx_in = nc.dram_tensor("idx_in", [128, T * 8], i16, kind="ExternalInput")
    out = nc.dram_tensor("out", [RPAD, C], f32, kind="ExternalOutput")

    # per-layer bounce pieces + gathered pieces (all fp8)
    l_wid = [HID, HID, HID, C_PAD]
    own_p = [[] for _ in range(NPIECE)]
    full_p = [[] for _ in range(NPIECE)]
    for l in range(4):
        for h in range(NPIECE):
            own_p[h].append(
                nc.dram_tensor(
                    f"own{'abc'[h]}{l}", [PIECE_ROWS[h], l_wid[l]], f8,
                    kind="Internal",
                )
            )
            full_p[h].append(
                nc.dram_tensor(
                    f"full{'abc'[h]}{l}",
                    [W_CORES * PIECE_ROWS[h], l_wid[l]], f8,
                    kind="Internal", addr_space="Shared",
                )
            )

    rg = [list(range(W_CORES))]

    with TileContext(nc) as tc:
        with (
            tc.tile_pool(name="const", bufs=1) as cpool,
            tc.tile_pool(name="work", bufs=2) as wpool,
            tc.tile_pool(name="psum", bufs=2, space="PSUM") as ppool,
        ):
            relu = mybir.ActivationFunctionType.Relu
            copyf = mybir.ActivationFunctionType.Copy
            dbl = mybir.MatmulPerfMode.DoubleRow

            # index-count registers for the call windows (per block-class)
            sizes = {CALL * 128}
            for b in range(MB):
                for h in range(NPIECE):
                    if ka[b][h] % CALL:
                        sizes.add((ka[b][h] % CALL) * 128)
            r_cnt = {n: nc.gpsimd.to_reg(n) for n in sizes}

            # ---- layer-1 GEMM inputs first (critical path) -------------------
            W1_sb = []
            for k0 in range(0, KFC, 6):
                k1 = min(k0 + 6, KFC)
                wg = cpool.tile([128, k1 - k0, HID], f8, tag=f"w1g{k0}")
                nc.sync.dma_start(out=wg[:], in_=W1l[:, k0:k1, :])
                W1_sb.append(wg)

            # resident G tiles, one per parity: [128, MB, HID] fp8.
            # Layer l's GEMM writes gbt[l%2]; layer l's local aggregation
            # reads it. (Layer-3 G4 occupies [:, :, :C_PAD].)
            gbt = [
                cpool.tile(
                    [128, MB, HID], f8, tag=f"gbt{p}", name=f"gbt{p}"
                )
                for p in range(2)
            ]

            def allgather(own, full):
                nc.gpsimd.collective_compute(
                    "AllGather",
                    mybir.AluOpType.bypass,
                    ins=[own[:]],
                    outs=[full[:]],
                    replica_groups=rg,
                )

            def store_own(lslot, m, wid):
                """DMA gbt slice for block m to its own piece rows."""
                h = 0 if m < 4 else 1
                r0 = m * 128 - PIECE_BASE[h]
                nc.sync.dma_start(
                    out=own_p[h][lslot][r0 : r0 + 128, :],
                    in_=gbt[lslot % 2][:, m, :wid],
                )

            def gemm_l1():
                # fp8 DoubleRow: 9 k-pair matmuls per block.
                for m in range(MB):
                    xm = wpool.tile([128, KFC, 128], f8, tag="xm", bufs=3)
                    nc.sync.dma_start(out=xm[:], in_=xT[m])
                    ps = ppool.tile([128, HID], f32, tag="gps")
                    for k in range(0, KFC, 2):
                        nc.tensor.matmul(
                            ps[:],
                            lhsT=xm[:, k : k + 2, :],
                            rhs=W1_sb[k // 6][:, k % 6 : k % 6 + 2, :],
                            start=(k == 0),
                            stop=(k == KFC - 2),
                            perf_mode=dbl,
                        )
                    nc.scalar.activation(
                        gbt[0][:, m, :], ps[:], copyf, scale=1.0 / W1_SCALE
                    )
                    store_own(0, m, HID)
                    if m == 3:
                        allgather(own_p[0][0], full_p[0][0])
                    elif m == MB - 1:
                        allgather(own_p[1][0], full_p[1][0])

            gemm_l1()

            # ---- remaining resident tensors (overlap the first collective) ---
            idx_sb = cpool.tile([128, T * 8], i16)
            nc.sync.dma_start(out=idx_sb[:], in_=idx_in[:])
            Sloc_sb = cpool.tile([128, MB * MB, 128], f8)
            nc.sync.dma_start(out=Sloc_sb[:], in_=Sloc_in[:])
            S_sb = cpool.tile([128, T, 128], f8)
            nc.sync.dma_start(out=S_sb[:], in_=S_in[:])
            W2_sb = cpool.tile([128, 4, HID], f8)
            nc.sync.dma_start(out=W2_sb[:], in_=W2l[:])
            W3_sb = cpool.tile([128, 4, HID], f8)
            nc.sync.dma_start(out=W3_sb[:], in_=W3l[:])
            W4_sb = cpool.tile([128, 4, C_PAD], f8)
            nc.sync.dma_start(out=W4_sb[:], in_=W4l[:])
            b_sb = []
            for nm, srcb in (("b1", b1r), ("b2", b2r), ("b3", b3r)):
                tle = cpool.tile([128, HID], f32, tag=f"bias_{nm}")
                nc.sync.dma_start(out=tle[:], in_=srcb[:])
                b_sb.append(tle)
            b4_sb = cpool.tile([128, C_PAD], f32)
            nc.sync.dma_start(out=b4_sb[:], in_=b4r[:])
            id_bf = cpool.tile([128, 128], bf16)
            make_identity(nc, id_bf[:])

            def issue_block_calls(b, h, fsrc, w, tiles):
                """Issue the gather calls for block b's piece-h chunk run;
                record tiles keyed by window-start chunk index."""
                t0 = tb_off[(b, h)]
                t1 = t0 + ka[b][h]
                for tw in range(t0, t1, CALL):
                    nk = min(CALL, t1 - tw)
                    msg = wpool.tile([128, CALL, w], f8, tag=f"msg{w}", bufs=24)
                    nc.gpsimd.dma_gather(
                        out_ap=msg[:, :nk, :],
                        in_ap=fsrc[:],
                        idxs_ap=idx_sb[:, tw * 8 : (tw + nk) * 8],
                        num_idxs=nk * 128,
                        num_idxs_reg=r_cnt[nk * 128],
                        elem_size=w,
                        queue_num=gq[0],
                    )
                    gq[0] = (gq[0] + 1) % N_QUEUES
                    tiles[tw] = msg

            def local_mms(ps, b, l, w, start):
                """Dense local aggregation for dst block b: 5 DoubleRow
                matmuls against the parity-resident G tile."""
                par = l % 2
                for m in range(0, MB, 2):
                    nc.tensor.matmul(
                        ps[:],
                        lhsT=Sloc_sb[:, b * MB + m : b * MB + m + 2, :],
                        rhs=gbt[par][:, m : m + 2, :w],
                        start=(start and m == 0),
                        stop=False,
                        perf_mode=dbl,
                    )

            def half_mms(ps, b, hs, tiles, start, stop):
                """Accumulate block b's chunks for the pieces in hs into ps,
                pairing consecutive chunks within a call window (DoubleRow)."""
                runs = []  # (t, tw, pair)
                for h in hs:
                    t0 = tb_off[(b, h)]
                    t1 = t0 + ka[b][h]
                    for tw in range(t0, t1, CALL):
                        nk = min(CALL, t1 - tw)
                        i = 0
                        while i < nk:
                            if i + 1 < nk:
                                runs.append((tw + i, tw, True))
                                i += 2
                            else:
                                runs.append((tw + i, tw, False))
                                i += 1
                for i, (t, tw, pair) in enumerate(runs):
                    msg = tiles[tw]
                    if pair:
                        nc.tensor.matmul(
                            ps[:],
                            lhsT=S_sb[:, t : t + 2, :],
                            rhs=msg[:, t - tw : t - tw + 2, :],
                            start=(start and i == 0),
                            stop=(stop and i == len(runs) - 1),
                            perf_mode=dbl,
                        )
                    else:
                        nc.tensor.matmul(
                            ps[:],
                            lhsT=S_sb[:, t, :],
                            rhs=msg[:, t - tw, :],
                            start=(start and i == 0),
                            stop=(stop and i == len(runs) - 1),
                        )

            lsm_t2 = {}

            def logsoftmax_block(ps, hacc, m):
                """Vector-only prefix of log_softmax for one block; the
                Exp/Ln passes are batched afterwards (and share one
                activation table, see _install_patches)."""
                lg = wpool.tile([128, C_PAD], f32, tag="lg")
                nc.vector.tensor_add(out=lg[:], in0=ps[:], in1=hacc[:])
                mx = wpool.tile([128, 1], f32, tag="mx")
                nc.vector.tensor_reduce(
                    out=mx[:], in_=lg[:, :C], axis=mybir.AxisListType.X,
                    op=mybir.AluOpType.max,
                )
                t2 = cpool.tile([128, C], f32, tag=f"t2_{m}", name=f"t2_{m}")
                nc.vector.tensor_scalar(
                    out=t2[:], in0=lg[:, :C], scalar1=mx[:], scalar2=None,
                    op0=mybir.AluOpType.subtract,
                )
                lsm_t2[m] = t2

            def logsoftmax_finish():
                e2s = {}
                for m in range(MB):
                    e2 = wpool.tile([128, C], f32, tag="e2", bufs=10)
                    nc.scalar.activation(
                        e2[:], lsm_t2[m][:], mybir.ActivationFunctionType.Exp
                    )
                    e2s[m] = e2
                sms = {}
                for m in range(MB):
                    sm = wpool.tile([128, 1], f32, tag="sm", bufs=10)
                    nc.vector.tensor_reduce(
                        out=sm[:], in_=e2s[m][:], axis=mybir.AxisListType.X,
                        op=mybir.AluOpType.add,
                    )
                    sms[m] = sm
                lss = {}
                for m in range(MB):
                    ls = wpool.tile([128, 1], f32, tag="ls", bufs=10)
                    nc.scalar.activation(
                        ls[:], sms[m][:], mybir.ActivationFunctionType.Ln
                    )
                    lss[m] = ls
                for m in range(MB):
                    o2 = wpool.tile([128, C], f32, tag="o2", bufs=4)
                    nc.vector.tensor_scalar(
                        out=o2[:], in0=lsm_t2[m][:], scalar1=lss[m][:],
                        scalar2=None, op0=mybir.AluOpType.subtract,
                    )
                    nc.sync.dma_start(
                        out=out[m * 128 : (m + 1) * 128, :], in_=o2[:]
                    )

            def block_tail(l, b, ps, hacc, bias_t, mode):
                """Bias + phase-A partial + relu + transpose + next-layer GEMM
                + store for one finished block (or the final classifier)."""
                if mode[0] == "final":
                    logsoftmax_block(ps, hacc, b)
                    return
                h0 = wpool.tile([128, HID], f32, tag="h0", bufs=3)
                nc.vector.tensor_add(out=h0[:], in0=ps[:], in1=hacc[:])
                hf = wpool.tile([128, HID], f32, tag="hf", bufs=3)
                nc.vector.tensor_add(out=hf[:], in0=h0[:], in1=bias_t[:])
                hb = wpool.tile([128, HID], bf16, tag="hb", bufs=3)
                nc.scalar.activation(hb[:], hf[:], relu)
                ht = wpool.tile([128, 4, 128], f8, tag="ht", bufs=4)
                for gg in range(4):
                    tp = ppool.tile([128, 128], bf16, tag="tps", bufs=1)
                    nc.tensor.transpose(
                        tp[:], hb[:, gg * 128 : (gg + 1) * 128], id_bf[:]
                    )
                    nc.vector.tensor_copy(out=ht[:, gg, :], in_=tp[:])
                _, w_sb, lnext = mode
                wid = l_wid[lnext]
                gp = ppool.tile([128, wid], f32, tag="gps")
                for k in range(0, 4, 2):
                    nc.tensor.matmul(
                        gp[:],
                        lhsT=ht[:, k : k + 2, :],
                        rhs=w_sb[:, k : k + 2, :],
                        start=(k == 0),
                        stop=(k == 2),
                        perf_mode=dbl,
                    )
                nc.scalar.activation(
                    gbt[lnext % 2][:, b, :wid], gp[:], copyf,
                    scale=1.0 / W_SCALE,
                )
                store_own(lnext, b, wid)

            def layer(l, bias_t, mode):
                """One fused layer: phase A accumulates local (SBUF-direct)
                + piece-a chunks into PSUM and spills; phase B adds piece-b
                and runs the per-block tail. The next layer's piece-a AG
                fires from tail 3, piece-b at layer end."""
                w = l_wid[l]
                pw = HID if mode[0] != "final" else C_PAD
                tiles = {}
                for b in range(MB):
                    issue_block_calls(b, 0, full_p[0][l], w, tiles)
                haccs = {}
                for b in range(MB):
                    psA = ppool.tile([128, pw], f32, tag="apsA", bufs=2)
                    local_mms(psA, b, l, w, True)
                    half_mms(psA, b, (0,), tiles, False, True)
                    # bf16 spill on DVE (2x rate, off the scalar engine);
                    # for the classifier layer fold the bias in for free
                    hc = wpool.tile([128, pw], bf16, tag="hacc", bufs=10)
                    if mode[0] == "final":
                        nc.vector.tensor_add(out=hc[:], in0=psA[:], in1=b4_sb[:])
                    else:
                        nc.vector.tensor_copy(out=hc[:], in_=psA[:])
                    haccs[b] = hc
                for b in range(MB):
                    issue_block_calls(b, 1, full_p[1][l], w, tiles)
                for b in range(MB):
                    psB = ppool.tile([128, pw], f32, tag="apsB", bufs=2)
                    half_mms(psB, b, (1,), tiles, True, True)
                    block_tail(l, b, psB, haccs[b], bias_t, mode)
                    if mode[0] != "final" and b == 3:
                        allgather(own_p[0][mode[2]], full_p[0][mode[2]])
                if mode[0] != "final":
                    allgather(own_p[1][mode[2]], full_p[1][mode[2]])
                else:
                    logsoftmax_finish()

            # ---- layers ----------------------------------------------------
            layer(0, b_sb[0], ("gemm", W2_sb, 1))
            layer(1, b_sb[1], ("gemm", W3_sb, 2))
            layer(2, b_sb[2], ("gemm", W4_sb, 3))
            layer(3, None, ("final",))

    nc.compile()
    return nc


# ----------------------------------------------------------------------------
# Entry point
# ----------------------------------------------------------------------------


def kernel(x, edge_index, batch, W1, b1, W2, b2, W3, b3, W4, b4, _trace=False):
    _install_drain_patch()
    from concourse.bass_utils import run_bass_kernel_spmd

    ka, in_maps = _prep_inputs(
        np.asarray(x, np.float32),
        np.asarray(edge_index),
        np.asarray(W1, np.float32), np.asarray(b1, np.float32),
        np.asarray(W2, np.float32), np.asarray(b2, np.float32),
        np.asarray(W3, np.float32), np.asarray(b3, np.float32),
        np.asarray(W4, np.float32), np.asarray(b4, np.float32),
    )
    key = tuple(ka)
    if key not in _cache:
        _cache[key] = _build(ka)
    nc = _cache[key]
    res = run_bass_kernel_spmd(
        nc, in_maps, core_ids=list(range(W_CORES)), trace=_trace
    )
    outp = np.concatenate(
        [res.results[c]["out"][:RPC] for c in range(W_CORES)], axis=0
    ).astype(np.float32)
    if _trace:
        return outp, res
    return outp
